# revision 2
# baseline (speedup 1.0000x reference)
"""Trainium2 Bass kernel for Transformer-XL style relative-position attention.

Problem: B=1, L=2048, D=1024, H=16 heads, dh=64. 8 NeuronCores.
Sharding: heads across cores (2 heads/core), QKV column-parallel,
output projection row-parallel.

I/O strategy (dispatch-overhead optimized):
  * ONE packed bf16 input blob per core (~3.4 MB): the core's 128-row
    D-slice of qT/kT/vT/posT, its head-sliced weights, and biases. One
    copy of every tensor is shipped in total across the 8 cores.
  * On device, the activation slices are AllGather'd (HBM collective)
    so every core sees the full qT/kT/vT/posT.
  * The 8 partial [L, D] f32 outputs from the row-parallel output
    projection are ReduceScatter'd so each core returns only its
    [L/8, D] f32 slice; the host just concatenates.

Per-core device program (scores computed TRANSPOSED, S^T[j, l]):
  1. Projections: qT/kT ([dout, L], lhsT=W^T slices, rhs=x^T), v ([L, dout]).
     Two q variants: q1 = scale*(q + bq + r_w_bias), q2 = scale*(q + bq + r_r_bias).
  2. pe^T[h] = r_kernel[h]^T @ pos_enc^T  ([dh, P]); cols beyond P zero-padded.
  3. rel[l, p] = q2_l . pe_p computed per l-tile, written to DRAM scratch SK with a
     *skewed* DRAM access pattern so SK[l, 128 + j] = rel[l, 2048 - l + j]
     (the _rel_shift). Read back with DMA-transpose (XBAR) as [j, l] tiles.
  4. S^T tile = kT-tile^T @ q1-chunk (+ rel via DVE add), P^T = exp(S^T) (ACT),
     diagonal blocks masked by an upper-triangular 0/1 mask after exp.
  5. AV: psum[l, 0:65] += P^T-subtile^T @ [v | 1]; col 64 = softmax denominator.
     Normalize with reciprocal * tensor_scalar.
  6. Output projection: attn tiles transposed via PE, matmul with Wo slice,
     + bo (bo packed only into core 0's blob), partials to DRAM f32.
  7. ReduceScatter(add) partials -> [L/8, D] f32 -> output.
"""
import sys

for p in ('/opt/trn_rl_repo', '/root/.axon_site/_ro/trn_rl_repo'):
    if p not in sys.path:
        sys.path.insert(0, p)

import numpy as np
import ml_dtypes

import bass_rust
import concourse.bass as bass
import concourse.mybir as mybir
import concourse.tile as tile
from concourse.masks import make_identity, make_upper_triangular

BF16 = mybir.dt.bfloat16
F32 = mybir.dt.float32
NPBF16 = ml_dtypes.bfloat16

L = 2048
D = 1024
H = 16
DH = 64
NCORES = 8
HPC = H // NCORES          # heads per core = 2
DLOC = HPC * DH            # per-core dout slice = 128
LLOC = L // NCORES         # per-core output rows = 256
P_POS = L + 1              # 2049
PE_W = 2176                # pe cols incl 127 zero-pad (covers masked diag region)
SKW = 2304                 # SK scratch row width: 128 left margin + 2048 + margin
SCALE = DH ** -0.5
NT = L // 128              # 16 l-tiles
NCH = L // 512             # 4 l-chunks

# packed blob layout (bf16 elements)
ACT_W = 8208               # 3*2048 + 2049 pos + 15 pad
ACT_E = 128 * ACT_W        # 1050624
W_E = D * DLOC             # 131072
WQOFF = ACT_E
WKOFF = WQOFF + W_E
WVOFF = WKOFF + W_E
RKOFF = WVOFF + W_E
WOOFF = RKOFF + W_E
BQ1OFF = WOOFF + W_E
BQ2OFF = BQ1OFF + DLOC
BKBOFF = BQ2OFF + DLOC
BVBOFF = BKBOFF + DLOC
BOOFF = BVBOFF + DLOC
TOT_E = BOOFF + D          # 1707520

# ---------------------------------------------------------------------------
# Tile/walrus compatibility patches (this walrus build accepts at most ONE
# sync wait per instruction; Tile can emit more). Hoist extras onto standalone
# EventSemaphore instructions, and split the kernel-tail drain's waits.
# ---------------------------------------------------------------------------
_PATCHED = False


def _apply_tile_patches():
    global _PATCHED
    if _PATCHED:
        return
    _PATCHED = True

    def _drain_and_barrier(self, tick_clock, wait_clock):
        nc = self.nc
        probe = mybir.InstNoOp(
            name="drain_wait_probe", ins=[], outs=[], engine=mybir.EngineType.SP
        )
        wait_clock.add_sem_waits(
            probe, bass_rust.ScopedClock({None: tick_clock.global_clock})
        )
        si = probe.sync_info
        waits = list(si.on_wait) if si is not None else []
        sems_by_name = {s.name: s for s in self.sems.allocated().values()}
        for w in waits:
            sem = sems_by_name.get(w.ant_name)
            assert sem is not None and w.wait_mode == "sem-ge-imm"
            nc.sync.wait_ge(sem, w.wait_value)
        nc.sync.drain()
        nc.all_engine_barrier()
        popped = nc._tile_sem_poison_stack.pop()
        assert popped is self._sem_poison
        nc.clear_and_free_semaphores(list(self.sems.allocated().values()))
        nc.all_engine_barrier()

    _orig_add = tile.TileContext._add_instruction
    ctr = [0]

    def _add_instruction(self, inst):
        si = inst.sync_info
        waits = list(si.on_wait) if si is not None else []
        if len(waits) > 1:
            best, order = {}, []
            for w in waits:
                k = w.ant_name
                if k not in best:
                    order.append(k)
                    best[k] = w
                elif (w.wait_value or 0) > (best[k].wait_value or 0):
                    best[k] = w
            waits = [best[k] for k in order]
            for w in waits[:-1]:
                ctr[0] += 1
                ev = mybir.InstEventSemaphore(
                    name=f"{inst.name}_hoistw{ctr[0]}",
                    ins=[],
                    outs=[],
                    engine=inst.engine,
                    sync_info=bass_rust.SyncInfo(on_wait=[w], on_update=[]),
                )
                _orig_add(self, ev)
            inst.sync_info = bass_rust.SyncInfo(
                on_wait=[waits[-1]], on_update=list(si.on_update)
            )
        _orig_add(self, inst)

    tile.TileContext._drain_and_barrier = _drain_and_barrier
    tile.TileContext._add_instruction = _add_instruction


# ---------------------------------------------------------------------------
# Device program
# ---------------------------------------------------------------------------
_CACHED_NC = None


def build_program():
    global _CACHED_NC
    if _CACHED_NC is not None:
        return _CACHED_NC
    _apply_tile_patches()

    nc = bass.Bass()
    blob = nc.dram_tensor("blob", [TOT_E], BF16, kind="ExternalInput")
    outp = nc.dram_tensor("outp", [LLOC, D], F32, kind="ExternalOutput")

    ND = D // 128  # 8 din tiles
    Exp = mybir.ActivationFunctionType.Exp
    Copy = mybir.ActivationFunctionType.Copy
    Ident = mybir.ActivationFunctionType.Identity
    ADD = mybir.AluOpType.add
    MULT = mybir.AluOpType.mult
    GROUP = [list(range(NCORES))]

    with tile.TileContext(nc) as tc:
        with (
            tc.tile_pool(name="constp", bufs=1) as constp,
            tc.tile_pool(name="acts", bufs=1) as acts,
            tc.tile_pool(name="vsp", bufs=1) as vsp,
            tc.tile_pool(name="ps", bufs=1, space="PSUM") as ps,
            tc.tile_pool(name="dramp", bufs=1, space="DRAM") as dramp,
        ):
            # ---- AllGather the activation slices ----
            agin = dramp.tile([128, ACT_W], BF16, name="agin")
            agout = dramp.tile([NCORES * 128, ACT_W], BF16, name="agout",
                               addr_space="Shared")
            nc.gpsimd.dma_start(
                out=agin[:, :],
                in_=bass.AP(blob, 0, [[ACT_W, 128], [1, ACT_W]]),
            )
            nc.gpsimd.collective_compute(
                "AllGather", mybir.AluOpType.bypass,
                replica_groups=GROUP,
                ins=[agin.opt()], outs=[agout.opt()],
            )

            # ---- constants (weights/biases from the blob) ----
            def load_w_tiles(off, name):
                ts = []
                for d in range(ND):
                    t = constp.tile([128, DLOC], BF16, name=f"{name}{d}")
                    nc.sync.dma_start(
                        out=t,
                        in_=bass.AP(blob, off + 128 * DLOC * d,
                                    [[DLOC, 128], [1, DLOC]]),
                    )
                    ts.append(t)
                return ts

            wq_t = load_w_tiles(WQOFF, "wq_t")
            wk_t = load_w_tiles(WKOFF, "wk_t")
            wv_t = load_w_tiles(WVOFF, "wv_t")
            rk_t = load_w_tiles(RKOFF, "rk_t")
            wo_h = []
            for h in range(HPC):
                t = constp.tile([DH, D], BF16, name=f"wo_h{h}")
                nc.sync.dma_start(
                    out=t,
                    in_=bass.AP(blob, WOOFF + DH * D * h, [[D, DH], [1, D]]),
                )
                wo_h.append(t)

            def load_bias(off, name):
                t = constp.tile([DLOC, 1], F32, name=name)
                nc.gpsimd.dma_start(
                    out=t, in_=bass.AP(blob, off, [[1, DLOC], [1, 1]])
                )
                return t

            bq1_t = load_bias(BQ1OFF, "bq1_t")
            bq2_t = load_bias(BQ2OFF, "bq2_t")
            bkb_t = load_bias(BKBOFF, "bkb_t")
            bvb_t = load_bias(BVBOFF, "bvb_t")

            # bo broadcast to [128, D] via PE (ones[1,128]^T outer bo[1,D])
            bo_row = constp.tile([1, D], BF16, name="bo_row")
            nc.sync.dma_start(
                out=bo_row, in_=bass.AP(blob, BOOFF, [[D, 1], [1, D]])
            )
            ones_c = constp.tile([1, 128], BF16, name="ones_c")
            nc.vector.memset(ones_c, 1.0)
            bo_full = constp.tile([128, D], F32, name="bo_full")
            for half in range(2):
                hs = slice(512 * half, 512 * (half + 1))
                pb = ps.tile([128, 512], F32, tag="cont", bufs=3, name="pb")
                nc.tensor.matmul(pb, ones_c, bo_row[:, hs],
                                 start=True, stop=True)
                nc.scalar.activation(bo_full[:, hs], pb, Copy)

            umask = constp.tile([128, 128], BF16, name="umask")
            make_upper_triangular(nc, umask, val=1.0)
            ident = constp.tile([128, 128], BF16, name="ident")
            make_identity(nc, ident)
            ident32 = constp.tile([128, 128], F32, name="ident32")
            make_identity(nc, ident32)

            # ---- persistent activations ----
            q1 = acts.tile([DLOC, L], BF16, name="q1")
            q2 = acts.tile([DLOC, L], BF16, name="q2")
            k1 = acts.tile([DLOC, L], BF16, name="k1")
            vpT = acts.tile([DLOC, L], BF16, name="vpT")
            peT = acts.tile([128, PE_W], BF16, name="peT")
            aT = [acts.tile([DH, L], BF16, name=f"aT{h}") for h in range(HPC)]
            recip_all = [
                acts.tile([128, NT], F32, name=f"recip{h}") for h in range(HPC)
            ]
            vS = [vsp.tile([128, 130], BF16, name=f"vS{j}") for j in range(NT)]
            sk = [
                dramp.tile([L, SKW], BF16, name=f"sk{h}") for h in range(HPC)
            ]
            rsin = dramp.tile([L, D], F32, name="rsin")
            rsout = dramp.tile([LLOC, D], F32, name="rsout")

            # ================= stage 1: projections =================
            with tc.tile_pool(name="inp", bufs=1) as inp:
                def load_in_tiles(col0, name, cols):
                    ts = []
                    for d in range(ND):
                        t = inp.tile([128, cols], BF16, name=f"{name}{d}")
                        eng = nc.sync if d % 2 == 0 else nc.scalar
                        eng.dma_start(
                            out=t,
                            in_=agout[128 * d:128 * (d + 1),
                                      col0:col0 + cols],
                        )
                        ts.append(t)
                    return ts

                qT_s = load_in_tiles(0, "qT_s", L)
                kT_s = load_in_tiles(L, "kT_s", L)
                vT_s = load_in_tiles(2 * L, "vT_s", L)
                posT_s = load_in_tiles(3 * L, "posT_s", P_POS)

                # projections grouped by tensor, matching DMA arrival order
                for c in range(NCH):
                    sl = slice(512 * c, 512 * (c + 1))
                    pq = ps.tile([128, 512], F32, tag="cont", bufs=3, name="pq")
                    for d in range(ND):
                        nc.tensor.matmul(
                            pq, wq_t[d], qT_s[d][:, sl],
                            start=(d == 0), stop=(d == ND - 1),
                        )
                    nc.scalar.activation(q1[:, sl], pq, Ident,
                                         bias=bq1_t, scale=SCALE)
                    nc.scalar.activation(q2[:, sl], pq, Ident,
                                         bias=bq2_t, scale=SCALE)
                for c in range(NCH):
                    sl = slice(512 * c, 512 * (c + 1))
                    pk = ps.tile([128, 512], F32, tag="cont", bufs=3, name="pk")
                    for d in range(ND):
                        nc.tensor.matmul(
                            pk, wk_t[d], kT_s[d][:, sl],
                            start=(d == 0), stop=(d == ND - 1),
                        )
                    nc.scalar.activation(k1[:, sl], pk, Ident, bias=bkb_t)
                for c in range(NCH):
                    sl = slice(512 * c, 512 * (c + 1))
                    pv = ps.tile([128, 512], F32, tag="cont", bufs=3, name="pv")
                    for d in range(ND):
                        nc.tensor.matmul(
                            pv, wv_t[d], vT_s[d][:, sl],
                            start=(d == 0), stop=(d == ND - 1),
                        )
                    nc.scalar.activation(vpT[:, sl], pv, Ident, bias=bvb_t)

                # pe^T (both heads stacked): rows 64h..64h+64 = head h
                pe_chunks = [(0, 512), (512, 512), (1024, 512), (1536, 512),
                             (2048, 1)]
                for (cs, cw) in pe_chunks:
                    pp = ps.tile([128, 512], F32, tag="cont", bufs=3,
                                 name="pp")
                    for d in range(ND):
                        nc.tensor.matmul(
                            pp[:, 0:cw], rk_t[d], posT_s[d][:, cs:cs + cw],
                            start=(d == 0), stop=(d == ND - 1),
                        )
                    nc.scalar.activation(peT[:, cs:cs + cw], pp[:, 0:cw], Copy)
                nc.vector.memset(peT[:, P_POS:PE_W], 0.0)

            # v transposes -> vS[t] = [v_h0 | 1 | v_h1 | 1]
            for t in range(NT):
                pvt = ps.tile([128, 128], BF16, tag="mm128", bufs=1,
                              name="pvt")
                nc.tensor.transpose(pvt, vpT[:, 128 * t:128 * (t + 1)], ident)
                nc.scalar.activation(vS[t][:, 0:DH], pvt[:, 0:DH], Copy)
                nc.scalar.activation(vS[t][:, 65:65 + DH], pvt[:, DH:DLOC],
                                     Copy)
                nc.vector.memset(vS[t][:, 64:65], 1.0)
                nc.vector.memset(vS[t][:, 129:130], 1.0)

            work = exit_stack_work = tc.tile_pool(name="work", bufs=1)
            work = work.__enter__()

            # ================= stage 2: rel -> skewed DRAM =================
            for t in range(NT):
                for h in range(HPC):
                    hsl = slice(DH * h, DH * (h + 1))
                    l0 = 128 * t
                    pmin = 1921 - l0
                    wrel = PE_W - pmin  # 128*t + 255
                    rel_sb = work.tile([128, PE_W], BF16, tag="rel_sb",
                                       bufs=3, name="rel_sb")
                    cs = 0
                    while cs < wrel:
                        cw = min(512, wrel - cs)
                        pr = ps.tile([128, 512], F32, tag="relp", bufs=2,
                                     name="pr")
                        nc.tensor.matmul(
                            pr[:, 0:cw], q2[hsl, l0:l0 + 128],
                            peT[hsl, pmin + cs:pmin + cs + cw],
                            start=True, stop=True,
                        )
                        nc.scalar.activation(
                            rel_sb[:, cs:cs + cw], pr[:, 0:cw], Copy
                        )
                        cs += cw
                    dst = bass.AP(
                        sk[h].tensor,
                        l0 * (SKW + 1) + pmin - 1920,
                        [[SKW + 1, 128], [1, wrel]],
                    )
                    nc.gpsimd.dma_start(out=dst, in_=rel_sb[:, 0:wrel])

            # ================= stage 3: scores/softmax/AV =================
            for h in range(HPC):
                hsl = slice(DH * h, DH * (h + 1))
                for c in range(NCH):
                    lc = 512 * c
                    nJ = 4 * (c + 1)
                    avp = ps.tile([65, 512], F32, tag="avT", bufs=2,
                                  name="avp")
                    pTs = []

                    def emit_av(J):
                        nc.tensor.matmul(
                            avp, vS[J][:, 65 * h:65 * (h + 1)], pTs[J],
                            start=(J == 0), stop=(J == nJ - 1),
                        )

                    for J in range(nJ):
                        j0 = 128 * J
                        col0 = max(0, j0 - lc)
                        wv_ = 512 - col0
                        pS = ps.tile([128, 512], F32, tag="cont", bufs=3,
                                     name="pS")
                        nc.tensor.matmul(
                            pS[:, 0:wv_], k1[hsl, j0:j0 + 128],
                            q1[hsl, lc + col0:lc + 512],
                            start=True, stop=True,
                        )
                        relT = work.tile([128, 512], BF16, tag="relT", bufs=6,
                                         name="relT")
                        nc.scalar.dma_start(
                            out=relT[:, 0:wv_],
                            in_=sk[h][lc + col0:lc + 512, 128 + j0:256 + j0],
                            transpose=True,
                        )
                        sc = work.tile([128, 512], F32, tag="sc", bufs=4,
                                       name="sc")
                        nc.vector.tensor_tensor(
                            sc[:, 0:wv_], pS[:, 0:wv_], relT[:, 0:wv_], ADD
                        )
                        pT = work.tile([128, 512], BF16, tag="pT", bufs=8,
                                       name="pT")
                        nc.scalar.activation(pT[:, col0:512], sc[:, 0:wv_],
                                             Exp)
                        if col0 > 0:
                            nc.gpsimd.memset(pT[:, 0:col0], 0.0)
                        if J >= 4 * c:
                            nc.gpsimd.tensor_tensor(
                                pT[:, col0:col0 + 128],
                                pT[:, col0:col0 + 128], umask, MULT,
                            )
                        pTs.append(pT)
                        emit_av(J)

                    # evict: rows 0..63 -> aT (bf16); denom row 64 -> f32
                    nc.scalar.activation(
                        aT[h][:, lc:lc + 512], avp[0:DH, :], Copy
                    )
                    den = work.tile([1, 512], F32, tag="den", bufs=1,
                                    name="den")
                    nc.scalar.activation(den, avp[DH:DH + 1, :], Copy)
                    pd = ps.tile([128, 4], F32, tag="mm128", bufs=1,
                                 name="pd")
                    for s in range(4):
                        nc.tensor.transpose(
                            pd[:, s:s + 1], den[:, 128 * s:128 * (s + 1)],
                            ident32[0:1, 0:1]
                        )
                    nc.vector.reciprocal(
                        recip_all[h][:, 4 * c:4 * c + 4], pd
                    )

            # ================= stage 4: output projection =================
            for t in range(NT):
                tsl = slice(128 * t, 128 * (t + 1))
                out_sb = work.tile([128, D], F32, tag="out_sb", bufs=2,
                                   name="out_sb")
                for oc in range(2):
                    osl = slice(512 * oc, 512 * (oc + 1))
                    po0 = ps.tile([128, 512], F32, tag="cont", bufs=3,
                                  name="po0")
                    nc.tensor.matmul(po0, aT[0][:, tsl], wo_h[0][:, osl],
                                     start=True, stop=True)
                    nc.vector.scalar_tensor_tensor(
                        out_sb[:, osl], po0, recip_all[0][:, t:t + 1],
                        bo_full[:, osl], MULT, ADD,
                    )
                    po1 = ps.tile([128, 512], F32, tag="cont", bufs=3,
                                  name="po1")
                    nc.tensor.matmul(po1, aT[1][:, tsl], wo_h[1][:, osl],
                                     start=True, stop=True)
                    nc.vector.scalar_tensor_tensor(
                        out_sb[:, osl], po1, recip_all[1][:, t:t + 1],
                        out_sb[:, osl], MULT, ADD,
                    )
                nc.sync.dma_start(out=rsin[tsl, :], in_=out_sb)

            # ============ stage 5: ReduceScatter -> output slice ============
            nc.gpsimd.collective_compute(
                "ReduceScatter", ADD,
                replica_groups=GROUP,
                ins=[rsin.opt()], outs=[rsout.opt()],
            )
            nc.sync.dma_start(out=outp[:, :], in_=rsout[:])

            exit_stack_work.__exit__(None, None, None)

    _CACHED_NC = nc
    return nc


# ---------------------------------------------------------------------------
# Host wrapper
# ---------------------------------------------------------------------------
def _prep_inputs(q, k, v, pos_enc, Wq, bq, Wk, bk, Wv, bv, Wo, bo,
                 r_w_bias, r_r_bias, r_kernel):
    q2d = np.asarray(q, np.float32).reshape(L, D)
    k2d = np.asarray(k, np.float32).reshape(L, D)
    v2d = np.asarray(v, np.float32).reshape(L, D)
    p2d = np.asarray(pos_enc, np.float32)
    rwb = np.asarray(r_w_bias, np.float32).reshape(H, DH)
    rrb = np.asarray(r_r_bias, np.float32).reshape(H, DH)
    Wq = np.asarray(Wq, np.float32)
    Wk = np.asarray(Wk, np.float32)
    Wv = np.asarray(Wv, np.float32)
    Wo = np.asarray(Wo, np.float32)
    rkn = np.asarray(r_kernel, np.float32)
    bq = np.asarray(bq, np.float32)
    bk = np.asarray(bk, np.float32)
    bv = np.asarray(bv, np.float32)
    bo = np.asarray(bo, np.float32)

    in_maps = []
    for c in range(NCORES):
        sl = slice(DLOC * c, DLOC * (c + 1))
        hsl = slice(HPC * c, HPC * (c + 1))
        blob = np.zeros(TOT_E, NPBF16)
        act = blob[:ACT_E].reshape(128, ACT_W)
        act[:, 0:L] = q2d[:, sl].T
        act[:, L:2 * L] = k2d[:, sl].T
        act[:, 2 * L:3 * L] = v2d[:, sl].T
        act[:, 3 * L:3 * L + P_POS] = p2d[:, sl].T
        blob[WQOFF:WQOFF + W_E] = Wq[sl].T.astype(NPBF16).ravel()
        blob[WKOFF:WKOFF + W_E] = Wk[sl].T.astype(NPBF16).ravel()
        blob[WVOFF:WVOFF + W_E] = Wv[sl].T.astype(NPBF16).ravel()
        rk_c = rkn[hsl]  # [2, D, DH]
        blob[RKOFF:RKOFF + W_E] = np.concatenate(
            [rk_c[0], rk_c[1]], axis=1).astype(NPBF16).ravel()
        blob[WOOFF:WOOFF + W_E] = Wo[:, sl].T.astype(NPBF16).ravel()
        bq_c = bq[sl]
        blob[BQ1OFF:BQ1OFF + DLOC] = (
            SCALE * (bq_c + rwb[hsl].reshape(DLOC))).astype(NPBF16)
        blob[BQ2OFF:BQ2OFF + DLOC] = (
            SCALE * (bq_c + rrb[hsl].reshape(DLOC))).astype(NPBF16)
        blob[BKBOFF:BKBOFF + DLOC] = bk[sl].astype(NPBF16)
        blob[BVBOFF:BVBOFF + DLOC] = bv[sl].astype(NPBF16)
        if c == 0:
            blob[BOOFF:BOOFF + D] = bo.astype(NPBF16)
        in_maps.append({"blob": blob})
    return in_maps


# ---------------------------------------------------------------------------
# Cached sharded PJRT executable (built once per process)
# ---------------------------------------------------------------------------
_CACHED_FN = None


def _get_fn():
    global _CACHED_FN
    if _CACHED_FN is not None:
        return _CACHED_FN
    import jax
    from jax.sharding import Mesh, PartitionSpec
    from jax.experimental.shard_map import shard_map
    from concourse import bass2jax

    nc = build_program()
    bass2jax.install_neuronx_cc_hook()
    partition_name = (
        nc.partition_id_tensor.name if nc.partition_id_tensor else None
    )
    in_names, out_names, out_avals, zero_shapes = [], [], [], []
    for alloc in nc.m.functions[0].allocations:
        if not isinstance(alloc, mybir.MemoryLocationSet):
            continue
        name = alloc.memorylocations[0].name
        if alloc.kind == "ExternalInput":
            if name != partition_name:
                in_names.append(name)
        elif alloc.kind == "ExternalOutput":
            shape = tuple(alloc.tensor_shape)
            dtype = mybir.dt.np(alloc.dtype)
            out_names.append(name)
            out_avals.append(jax.core.ShapedArray(shape, dtype))
            zero_shapes.append((shape, dtype))
    n_params = len(in_names)
    n_outs = len(out_avals)
    all_in_names = list(in_names) + list(out_names)
    if partition_name is not None:
        all_in_names.append(partition_name)

    def _body(*args):
        operands = list(args)
        if partition_name is not None:
            operands.append(bass2jax.partition_id_tensor())
        outs = bass2jax._bass_exec_p.bind(
            *operands,
            out_avals=tuple(out_avals),
            in_names=tuple(all_in_names),
            out_names=tuple(out_names),
            lowering_input_output_aliases=(),
            sim_require_finite=True,
            sim_require_nnan=True,
            nc=nc,
        )
        return tuple(outs)

    donate = tuple(range(n_params, n_params + n_outs))
    devices = jax.devices()[:NCORES]
    mesh = Mesh(np.asarray(devices), ("core",))
    in_specs = (PartitionSpec("core"),) * (n_params + n_outs)
    out_specs = (PartitionSpec("core"),) * n_outs
    fn = jax.jit(
        shard_map(_body, mesh=mesh, in_specs=in_specs,
                  out_specs=out_specs, check_rep=False),
        donate_argnums=donate,
        keep_unused=True,
    )
    _CACHED_FN = (fn, in_names, out_names, zero_shapes)
    return _CACHED_FN


def kernel(**inputs):
    fn, in_names, out_names, zero_shapes = _get_fn()
    in_maps = _prep_inputs(**inputs)
    concat_in = [
        np.concatenate([m[name] for m in in_maps], axis=0)
        for name in in_names
    ]
    zeros = [
        np.zeros((NCORES * s[0], *s[1:]), dt) for (s, dt) in zero_shapes
    ]
    outs = fn(*concat_in, *zeros)
    out = np.asarray(outs[out_names.index("outp")])  # [L, D] f32
    return out.reshape(1, L, D).astype(np.float32)


# revision 3
# speedup vs baseline: 1.4518x; 1.4518x over previous
"""Trainium2 Bass kernel for Transformer-XL style relative-position attention.

Problem: B=1, L=2048, D=1024, H=16 heads, dh=64. 8 NeuronCores.
Sharding: heads across cores (2 heads/core), QKV column-parallel,
output projection row-parallel.

I/O strategy (dispatch-overhead optimized):
  * ONE packed bf16 input blob per core (~3.4 MB): the core's 128-row
    D-slice of qT/kT/vT/posT, its head-sliced weights, and biases. One
    copy of every tensor is shipped in total across the 8 cores.
  * On device, the activation slices are AllGather'd (HBM collective)
    so every core sees the full qT/kT/vT/posT.
  * The 8 partial [L, D] f32 outputs from the row-parallel output
    projection are ReduceScatter'd so each core returns only its
    [L/8, D] f32 slice; the host just concatenates.

Per-core device program (scores computed TRANSPOSED, S^T[j, l]):
  1. Projections: qT/kT ([dout, L], lhsT=W^T slices, rhs=x^T), v ([L, dout]).
     Two q variants: q1 = scale*(q + bq + r_w_bias), q2 = scale*(q + bq + r_r_bias).
  2. pe^T[h] = r_kernel[h]^T @ pos_enc^T  ([dh, P]); cols beyond P zero-padded.
  3. rel[l, p] = q2_l . pe_p computed per l-tile, written to DRAM scratch SK with a
     *skewed* DRAM access pattern so SK[l, 128 + j] = rel[l, 2048 - l + j]
     (the _rel_shift). Read back with DMA-transpose (XBAR) as [j, l] tiles.
  4. S^T tile = kT-tile^T @ q1-chunk (+ rel via DVE add), P^T = exp(S^T) (ACT),
     diagonal blocks masked by an upper-triangular 0/1 mask after exp.
  5. AV: psum[l, 0:65] += P^T-subtile^T @ [v | 1]; col 64 = softmax denominator.
     Normalize with reciprocal * tensor_scalar.
  6. Output projection: attn tiles transposed via PE, matmul with Wo slice,
     + bo (bo packed only into core 0's blob), partials to DRAM f32.
  7. ReduceScatter(add) partials -> [L/8, D] f32 -> output.
"""
import sys

for p in ('/opt/trn_rl_repo', '/root/.axon_site/_ro/trn_rl_repo'):
    if p not in sys.path:
        sys.path.insert(0, p)

import numpy as np
import ml_dtypes

import bass_rust
import concourse.bass as bass
import concourse.mybir as mybir
import concourse.tile as tile
from concourse.masks import make_identity, make_upper_triangular

BF16 = mybir.dt.bfloat16
F32 = mybir.dt.float32
NPBF16 = ml_dtypes.bfloat16

L = 2048
D = 1024
H = 16
DH = 64
NCORES = 8
HPC = H // NCORES          # heads per core = 2
DLOC = HPC * DH            # per-core dout slice = 128
LLOC = L // NCORES         # per-core output rows = 256
P_POS = L + 1              # 2049
PE_W = 2176                # pe cols incl 127 zero-pad (covers masked diag region)
SKW = 2304                 # SK scratch row width: 128 left margin + 2048 + margin
SCALE = DH ** -0.5
NT = L // 128              # 16 l-tiles
NCH = L // 512             # 4 l-chunks

# packed blob layout (bf16 elements)
ACT_W = 8208               # 3*2048 + 2049 pos + 15 pad
ACT_E = 128 * ACT_W        # 1050624
W_E = D * DLOC             # 131072
WQOFF = ACT_E
WKOFF = WQOFF + W_E
WVOFF = WKOFF + W_E
RKOFF = WVOFF + W_E
WOOFF = RKOFF + W_E
BQ1OFF = WOOFF + W_E
BQ2OFF = BQ1OFF + DLOC
BKBOFF = BQ2OFF + DLOC
BVBOFF = BKBOFF + DLOC
BOOFF = BVBOFF + DLOC
TOT_E = BOOFF + D          # 1707520

# ---------------------------------------------------------------------------
# Tile/walrus compatibility patches (this walrus build accepts at most ONE
# sync wait per instruction; Tile can emit more). Hoist extras onto standalone
# EventSemaphore instructions, and split the kernel-tail drain's waits.
# ---------------------------------------------------------------------------
_PATCHED = False


def _apply_tile_patches():
    global _PATCHED
    if _PATCHED:
        return
    _PATCHED = True

    def _drain_and_barrier(self, tick_clock, wait_clock):
        nc = self.nc
        probe = mybir.InstNoOp(
            name="drain_wait_probe", ins=[], outs=[], engine=mybir.EngineType.SP
        )
        wait_clock.add_sem_waits(
            probe, bass_rust.ScopedClock({None: tick_clock.global_clock})
        )
        si = probe.sync_info
        waits = list(si.on_wait) if si is not None else []
        sems_by_name = {s.name: s for s in self.sems.allocated().values()}
        for w in waits:
            sem = sems_by_name.get(w.ant_name)
            assert sem is not None and w.wait_mode == "sem-ge-imm"
            nc.sync.wait_ge(sem, w.wait_value)
        nc.sync.drain()
        nc.all_engine_barrier()
        popped = nc._tile_sem_poison_stack.pop()
        assert popped is self._sem_poison
        nc.clear_and_free_semaphores(list(self.sems.allocated().values()))
        nc.all_engine_barrier()

    _orig_add = tile.TileContext._add_instruction
    ctr = [0]

    def _add_instruction(self, inst):
        si = inst.sync_info
        waits = list(si.on_wait) if si is not None else []
        if len(waits) > 1:
            best, order = {}, []
            for w in waits:
                k = w.ant_name
                if k not in best:
                    order.append(k)
                    best[k] = w
                elif (w.wait_value or 0) > (best[k].wait_value or 0):
                    best[k] = w
            waits = [best[k] for k in order]
            for w in waits[:-1]:
                ctr[0] += 1
                ev = mybir.InstEventSemaphore(
                    name=f"{inst.name}_hoistw{ctr[0]}",
                    ins=[],
                    outs=[],
                    engine=inst.engine,
                    sync_info=bass_rust.SyncInfo(on_wait=[w], on_update=[]),
                )
                _orig_add(self, ev)
            inst.sync_info = bass_rust.SyncInfo(
                on_wait=[waits[-1]], on_update=list(si.on_update)
            )
        _orig_add(self, inst)

    tile.TileContext._drain_and_barrier = _drain_and_barrier
    tile.TileContext._add_instruction = _add_instruction


# ---------------------------------------------------------------------------
# Device program
# ---------------------------------------------------------------------------
_CACHED_NC = None


def build_program():
    global _CACHED_NC
    if _CACHED_NC is not None:
        return _CACHED_NC
    _apply_tile_patches()

    nc = bass.Bass()
    blob = nc.dram_tensor("blob", [TOT_E], BF16, kind="ExternalInput")
    outp = nc.dram_tensor("outp", [LLOC, D], F32, kind="ExternalOutput")

    ND = D // 128  # 8 din tiles
    Exp = mybir.ActivationFunctionType.Exp
    Copy = mybir.ActivationFunctionType.Copy
    Ident = mybir.ActivationFunctionType.Identity
    ADD = mybir.AluOpType.add
    MULT = mybir.AluOpType.mult
    GROUP = [list(range(NCORES))]

    with tile.TileContext(nc) as tc:
        with (
            tc.tile_pool(name="constp", bufs=1) as constp,
            tc.tile_pool(name="acts", bufs=1) as acts,
            tc.tile_pool(name="vsp", bufs=1) as vsp,
            tc.tile_pool(name="ps", bufs=1, space="PSUM") as ps,
            tc.tile_pool(name="dramp", bufs=1, space="DRAM") as dramp,
        ):
            # ---- AllGather the activation slices ----
            agin = dramp.tile([128, ACT_W], BF16, name="agin")
            agout = dramp.tile([NCORES * 128, ACT_W], BF16, name="agout",
                               addr_space="Shared")
            nc.gpsimd.dma_start(
                out=agin[:, :],
                in_=bass.AP(blob, 0, [[ACT_W, 128], [1, ACT_W]]),
            )
            nc.gpsimd.collective_compute(
                "AllGather", mybir.AluOpType.bypass,
                replica_groups=GROUP,
                ins=[agin.opt()], outs=[agout.opt()],
            )

            # ---- constants (weights/biases from the blob) ----
            def load_w_tiles(off, name):
                ts = []
                for d in range(ND):
                    t = constp.tile([128, DLOC], BF16, name=f"{name}{d}")
                    nc.sync.dma_start(
                        out=t,
                        in_=bass.AP(blob, off + 128 * DLOC * d,
                                    [[DLOC, 128], [1, DLOC]]),
                    )
                    ts.append(t)
                return ts

            wq_t = load_w_tiles(WQOFF, "wq_t")
            wk_t = load_w_tiles(WKOFF, "wk_t")
            wv_t = load_w_tiles(WVOFF, "wv_t")
            rk_t = load_w_tiles(RKOFF, "rk_t")
            wo_h = []
            for h in range(HPC):
                t = constp.tile([DH, D], BF16, name=f"wo_h{h}")
                nc.sync.dma_start(
                    out=t,
                    in_=bass.AP(blob, WOOFF + DH * D * h, [[D, DH], [1, D]]),
                )
                wo_h.append(t)

            def load_bias(off, name):
                t = constp.tile([DLOC, 1], F32, name=name)
                nc.gpsimd.dma_start(
                    out=t, in_=bass.AP(blob, off, [[1, DLOC], [1, 1]])
                )
                return t

            bq1_t = load_bias(BQ1OFF, "bq1_t")
            bq2_t = load_bias(BQ2OFF, "bq2_t")
            bkb_t = load_bias(BKBOFF, "bkb_t")
            bvb_t = load_bias(BVBOFF, "bvb_t")

            # bo broadcast to [128, D] via PE (ones[1,128]^T outer bo[1,D])
            bo_row = constp.tile([1, D], BF16, name="bo_row")
            nc.sync.dma_start(
                out=bo_row, in_=bass.AP(blob, BOOFF, [[D, 1], [1, D]])
            )
            ones_c = constp.tile([1, 128], BF16, name="ones_c")
            nc.vector.memset(ones_c, 1.0)
            bo_full = constp.tile([128, D], F32, name="bo_full")
            for half in range(2):
                hs = slice(512 * half, 512 * (half + 1))
                pb = ps.tile([128, 512], F32, tag="cont", bufs=3, name="pb")
                nc.tensor.matmul(pb, ones_c, bo_row[:, hs],
                                 start=True, stop=True)
                nc.scalar.activation(bo_full[:, hs], pb, Copy)

            umask = constp.tile([128, 128], BF16, name="umask")
            make_upper_triangular(nc, umask, val=1.0)
            ident = constp.tile([128, 128], BF16, name="ident")
            make_identity(nc, ident)
            ident32 = constp.tile([128, 128], F32, name="ident32")
            make_identity(nc, ident32)

            # ---- persistent activations ----
            q1 = acts.tile([DLOC, L], BF16, name="q1")
            q2 = acts.tile([DLOC, L], BF16, name="q2")
            k1 = acts.tile([DLOC, L], BF16, name="k1")
            vpT = acts.tile([DLOC, L], BF16, name="vpT")
            peT = acts.tile([128, PE_W], BF16, name="peT")
            aT = [acts.tile([DH, L], BF16, name=f"aT{h}") for h in range(HPC)]
            recip_all = [
                acts.tile([128, NT], F32, name=f"recip{h}") for h in range(HPC)
            ]
            vS = [vsp.tile([128, 130], BF16, name=f"vS{j}") for j in range(NT)]
            sk = [
                dramp.tile([L, SKW], BF16, name=f"sk{h}") for h in range(HPC)
            ]
            rsin = dramp.tile([L, D], F32, name="rsin")
            rsout = dramp.tile([LLOC, D], F32, name="rsout")

            # ================= stage 1: projections =================
            with tc.tile_pool(name="inp", bufs=1) as inp:
                def load_in_tiles(col0, name, cols):
                    ts = []
                    for d in range(ND):
                        t = inp.tile([128, cols], BF16, name=f"{name}{d}")
                        eng = nc.sync if d % 2 == 0 else nc.scalar
                        eng.dma_start(
                            out=t,
                            in_=agout[128 * d:128 * (d + 1),
                                      col0:col0 + cols],
                        )
                        ts.append(t)
                    return ts

                qT_s = load_in_tiles(0, "qT_s", L)
                kT_s = load_in_tiles(L, "kT_s", L)
                vT_s = load_in_tiles(2 * L, "vT_s", L)
                posT_s = load_in_tiles(3 * L, "posT_s", P_POS)

                # projections grouped by tensor, matching DMA arrival order
                for c in range(NCH):
                    sl = slice(512 * c, 512 * (c + 1))
                    pq = ps.tile([128, 512], F32, tag="cont", bufs=3, name="pq")
                    for d in range(ND):
                        nc.tensor.matmul(
                            pq, wq_t[d], qT_s[d][:, sl],
                            start=(d == 0), stop=(d == ND - 1),
                        )
                    nc.scalar.activation(q1[:, sl], pq, Ident,
                                         bias=bq1_t, scale=SCALE)
                    nc.scalar.activation(q2[:, sl], pq, Ident,
                                         bias=bq2_t, scale=SCALE)
                for c in range(NCH):
                    sl = slice(512 * c, 512 * (c + 1))
                    pk = ps.tile([128, 512], F32, tag="cont", bufs=3, name="pk")
                    for d in range(ND):
                        nc.tensor.matmul(
                            pk, wk_t[d], kT_s[d][:, sl],
                            start=(d == 0), stop=(d == ND - 1),
                        )
                    nc.scalar.activation(k1[:, sl], pk, Ident, bias=bkb_t)
                for c in range(NCH):
                    sl = slice(512 * c, 512 * (c + 1))
                    pv = ps.tile([128, 512], F32, tag="cont", bufs=3, name="pv")
                    for d in range(ND):
                        nc.tensor.matmul(
                            pv, wv_t[d], vT_s[d][:, sl],
                            start=(d == 0), stop=(d == ND - 1),
                        )
                    nc.scalar.activation(vpT[:, sl], pv, Ident, bias=bvb_t)

                # pe^T (both heads stacked): rows 64h..64h+64 = head h
                pe_chunks = [(0, 512), (512, 512), (1024, 512), (1536, 512),
                             (2048, 1)]
                for (cs, cw) in pe_chunks:
                    pp = ps.tile([128, 512], F32, tag="cont", bufs=3,
                                 name="pp")
                    for d in range(ND):
                        nc.tensor.matmul(
                            pp[:, 0:cw], rk_t[d], posT_s[d][:, cs:cs + cw],
                            start=(d == 0), stop=(d == ND - 1),
                        )
                    nc.scalar.activation(peT[:, cs:cs + cw], pp[:, 0:cw], Copy)
                nc.vector.memset(peT[:, P_POS:PE_W], 0.0)

            # v transposes -> vS[t] = [v_h0 | 1 | v_h1 | 1]
            for t in range(NT):
                pvt = ps.tile([128, 128], BF16, tag="mm128", bufs=1,
                              name="pvt")
                nc.tensor.transpose(pvt, vpT[:, 128 * t:128 * (t + 1)], ident)
                nc.scalar.activation(vS[t][:, 0:DH], pvt[:, 0:DH], Copy)
                nc.scalar.activation(vS[t][:, 65:65 + DH], pvt[:, DH:DLOC],
                                     Copy)
                nc.vector.memset(vS[t][:, 64:65], 1.0)
                nc.vector.memset(vS[t][:, 129:130], 1.0)

            work = exit_stack_work = tc.tile_pool(name="work", bufs=1)
            work = work.__enter__()

            # ================= stage 2: rel -> skewed DRAM =================
            for t in range(NT):
                for h in range(HPC):
                    hsl = slice(DH * h, DH * (h + 1))
                    l0 = 128 * t
                    pmin = 1921 - l0
                    wrel = PE_W - pmin  # 128*t + 255
                    rel_sb = work.tile([128, PE_W], BF16, tag="rel_sb",
                                       bufs=3, name="rel_sb")
                    cs = 0
                    while cs < wrel:
                        cw = min(512, wrel - cs)
                        pr = ps.tile([128, 512], F32, tag="relp", bufs=2,
                                     name="pr")
                        nc.tensor.matmul(
                            pr[:, 0:cw], q2[hsl, l0:l0 + 128],
                            peT[hsl, pmin + cs:pmin + cs + cw],
                            start=True, stop=True,
                        )
                        nc.scalar.activation(
                            rel_sb[:, cs:cs + cw], pr[:, 0:cw], Copy
                        )
                        cs += cw
                    dst = bass.AP(
                        sk[h].tensor,
                        l0 * (SKW + 1) + pmin - 1920,
                        [[SKW + 1, 128], [1, wrel]],
                    )
                    nc.gpsimd.dma_start(out=dst, in_=rel_sb[:, 0:wrel])

            # ================= stage 3: scores/softmax/AV =================
            for h in range(HPC):
                hsl = slice(DH * h, DH * (h + 1))
                for c in range(NCH):
                    lc = 512 * c
                    nJ = 4 * (c + 1)
                    avp = ps.tile([65, 512], F32, tag="avT", bufs=2,
                                  name="avp")
                    pTs = []

                    def emit_av(J):
                        nc.tensor.matmul(
                            avp, vS[J][:, 65 * h:65 * (h + 1)], pTs[J],
                            start=(J == 0), stop=(J == nJ - 1),
                        )

                    for J in range(nJ):
                        j0 = 128 * J
                        col0 = max(0, j0 - lc)
                        wv_ = 512 - col0
                        pS = ps.tile([128, 512], F32, tag="cont", bufs=3,
                                     name="pS")
                        nc.tensor.matmul(
                            pS[:, 0:wv_], k1[hsl, j0:j0 + 128],
                            q1[hsl, lc + col0:lc + 512],
                            start=True, stop=True,
                        )
                        relT = work.tile([128, 512], BF16, tag="relT", bufs=6,
                                         name="relT")
                        nc.scalar.dma_start(
                            out=relT[:, 0:wv_],
                            in_=sk[h][lc + col0:lc + 512, 128 + j0:256 + j0],
                            transpose=True,
                        )
                        sc = work.tile([128, 512], F32, tag="sc", bufs=4,
                                       name="sc")
                        nc.vector.tensor_tensor(
                            sc[:, 0:wv_], pS[:, 0:wv_], relT[:, 0:wv_], ADD
                        )
                        pT = work.tile([128, 512], BF16, tag="pT", bufs=8,
                                       name="pT")
                        nc.scalar.activation(pT[:, col0:512], sc[:, 0:wv_],
                                             Exp)
                        if col0 > 0:
                            nc.gpsimd.memset(pT[:, 0:col0], 0.0)
                        if J >= 4 * c:
                            nc.gpsimd.tensor_tensor(
                                pT[:, col0:col0 + 128],
                                pT[:, col0:col0 + 128], umask, MULT,
                            )
                        pTs.append(pT)
                        emit_av(J)

                    # evict: rows 0..63 -> aT (bf16); denom row 64 -> f32
                    nc.scalar.activation(
                        aT[h][:, lc:lc + 512], avp[0:DH, :], Copy
                    )
                    den = work.tile([1, 512], F32, tag="den", bufs=1,
                                    name="den")
                    nc.scalar.activation(den, avp[DH:DH + 1, :], Copy)
                    pd = ps.tile([128, 4], F32, tag="mm128", bufs=1,
                                 name="pd")
                    for s in range(4):
                        nc.tensor.transpose(
                            pd[:, s:s + 1], den[:, 128 * s:128 * (s + 1)],
                            ident32[0:1, 0:1]
                        )
                    nc.vector.reciprocal(
                        recip_all[h][:, 4 * c:4 * c + 4], pd
                    )

            # ================= stage 4: output projection =================
            for t in range(NT):
                tsl = slice(128 * t, 128 * (t + 1))
                out_sb = work.tile([128, D], F32, tag="out_sb", bufs=2,
                                   name="out_sb")
                for oc in range(2):
                    osl = slice(512 * oc, 512 * (oc + 1))
                    po0 = ps.tile([128, 512], F32, tag="cont", bufs=3,
                                  name="po0")
                    nc.tensor.matmul(po0, aT[0][:, tsl], wo_h[0][:, osl],
                                     start=True, stop=True)
                    nc.vector.scalar_tensor_tensor(
                        out_sb[:, osl], po0, recip_all[0][:, t:t + 1],
                        bo_full[:, osl], MULT, ADD,
                    )
                    po1 = ps.tile([128, 512], F32, tag="cont", bufs=3,
                                  name="po1")
                    nc.tensor.matmul(po1, aT[1][:, tsl], wo_h[1][:, osl],
                                     start=True, stop=True)
                    nc.vector.scalar_tensor_tensor(
                        out_sb[:, osl], po1, recip_all[1][:, t:t + 1],
                        out_sb[:, osl], MULT, ADD,
                    )
                nc.sync.dma_start(out=rsin[tsl, :], in_=out_sb)

            # ============ stage 5: ReduceScatter -> output slice ============
            nc.gpsimd.collective_compute(
                "ReduceScatter", ADD,
                replica_groups=GROUP,
                ins=[rsin.opt()], outs=[rsout.opt()],
            )
            nc.sync.dma_start(out=outp[:, :], in_=rsout[:])

            exit_stack_work.__exit__(None, None, None)

    _CACHED_NC = nc
    return nc


# ---------------------------------------------------------------------------
# Host wrapper
# ---------------------------------------------------------------------------
def _prep_inputs(q, k, v, pos_enc, Wq, bq, Wk, bk, Wv, bv, Wo, bo,
                 r_w_bias, r_r_bias, r_kernel):
    q2d = np.asarray(q, np.float32).reshape(L, D)
    k2d = np.asarray(k, np.float32).reshape(L, D)
    v2d = np.asarray(v, np.float32).reshape(L, D)
    p2d = np.asarray(pos_enc, np.float32)
    rwb = np.asarray(r_w_bias, np.float32).reshape(H, DH)
    rrb = np.asarray(r_r_bias, np.float32).reshape(H, DH)
    Wq = np.asarray(Wq, np.float32)
    Wk = np.asarray(Wk, np.float32)
    Wv = np.asarray(Wv, np.float32)
    Wo = np.asarray(Wo, np.float32)
    rkn = np.asarray(r_kernel, np.float32)
    bq = np.asarray(bq, np.float32)
    bk = np.asarray(bk, np.float32)
    bv = np.asarray(bv, np.float32)
    bo = np.asarray(bo, np.float32)

    in_maps = []
    for c in range(NCORES):
        sl = slice(DLOC * c, DLOC * (c + 1))
        hsl = slice(HPC * c, HPC * (c + 1))
        blob = np.zeros(TOT_E, NPBF16)
        act = blob[:ACT_E].reshape(128, ACT_W)
        act[:, 0:L] = q2d[:, sl].T
        act[:, L:2 * L] = k2d[:, sl].T
        act[:, 2 * L:3 * L] = v2d[:, sl].T
        act[:, 3 * L:3 * L + P_POS] = p2d[:, sl].T
        blob[WQOFF:WQOFF + W_E] = Wq[sl].T.astype(NPBF16).ravel()
        blob[WKOFF:WKOFF + W_E] = Wk[sl].T.astype(NPBF16).ravel()
        blob[WVOFF:WVOFF + W_E] = Wv[sl].T.astype(NPBF16).ravel()
        rk_c = rkn[hsl]  # [2, D, DH]
        blob[RKOFF:RKOFF + W_E] = np.concatenate(
            [rk_c[0], rk_c[1]], axis=1).astype(NPBF16).ravel()
        blob[WOOFF:WOOFF + W_E] = Wo[:, sl].T.astype(NPBF16).ravel()
        bq_c = bq[sl]
        blob[BQ1OFF:BQ1OFF + DLOC] = (
            SCALE * (bq_c + rwb[hsl].reshape(DLOC))).astype(NPBF16)
        blob[BQ2OFF:BQ2OFF + DLOC] = (
            SCALE * (bq_c + rrb[hsl].reshape(DLOC))).astype(NPBF16)
        blob[BKBOFF:BKBOFF + DLOC] = bk[sl].astype(NPBF16)
        blob[BVBOFF:BVBOFF + DLOC] = bv[sl].astype(NPBF16)
        if c == 0:
            blob[BOOFF:BOOFF + D] = bo.astype(NPBF16)
        in_maps.append({"blob": blob})
    return in_maps


# ---------------------------------------------------------------------------
# Cached sharded PJRT executable (built once per process)
# ---------------------------------------------------------------------------
_CACHED_FN = None


def _get_fn():
    global _CACHED_FN
    if _CACHED_FN is not None:
        return _CACHED_FN
    import jax
    from jax.sharding import Mesh, PartitionSpec
    from jax.experimental.shard_map import shard_map
    from concourse import bass2jax

    nc = build_program()
    bass2jax.install_neuronx_cc_hook()
    partition_name = (
        nc.partition_id_tensor.name if nc.partition_id_tensor else None
    )
    in_names, out_names, out_avals, zero_shapes = [], [], [], []
    for alloc in nc.m.functions[0].allocations:
        if not isinstance(alloc, mybir.MemoryLocationSet):
            continue
        name = alloc.memorylocations[0].name
        if alloc.kind == "ExternalInput":
            if name != partition_name:
                in_names.append(name)
        elif alloc.kind == "ExternalOutput":
            shape = tuple(alloc.tensor_shape)
            dtype = mybir.dt.np(alloc.dtype)
            out_names.append(name)
            out_avals.append(jax.core.ShapedArray(shape, dtype))
            zero_shapes.append((shape, dtype))
    n_params = len(in_names)
    n_outs = len(out_avals)
    all_in_names = list(in_names) + list(out_names)
    if partition_name is not None:
        all_in_names.append(partition_name)

    def _body(*args):
        operands = list(args)
        if partition_name is not None:
            operands.append(bass2jax.partition_id_tensor())
        outs = bass2jax._bass_exec_p.bind(
            *operands,
            out_avals=tuple(out_avals),
            in_names=tuple(all_in_names),
            out_names=tuple(out_names),
            lowering_input_output_aliases=(),
            sim_require_finite=True,
            sim_require_nnan=True,
            nc=nc,
        )
        return tuple(outs)

    donate = tuple(range(n_params, n_params + n_outs))
    devices = jax.devices()[:NCORES]
    mesh = Mesh(np.asarray(devices), ("core",))
    in_specs = (PartitionSpec("core"),) * (n_params + n_outs)
    out_specs = (PartitionSpec("core"),) * n_outs
    fn = jax.jit(
        shard_map(_body, mesh=mesh, in_specs=in_specs,
                  out_specs=out_specs, check_rep=False),
        donate_argnums=donate,
        keep_unused=True,
    )
    _CACHED_FN = (fn, in_names, out_names, zero_shapes)
    return _CACHED_FN


def kernel(**inputs):
    global _CACHED_FN
    in_maps = _prep_inputs(**inputs)
    last_exc = None
    for attempt in range(2):
        try:
            fn, in_names, out_names, zero_shapes = _get_fn()
            concat_in = [
                np.concatenate([m[name] for m in in_maps], axis=0)
                for name in in_names
            ]
            zeros = [
                np.zeros((NCORES * s[0], *s[1:]), dt)
                for (s, dt) in zero_shapes
            ]
            outs = fn(*concat_in, *zeros)
            out = np.asarray(outs[out_names.index("outp")])  # [L, D] f32
            return out.reshape(1, L, D).astype(np.float32)
        except Exception as e:  # transient axon/mesh hiccup: rebuild once
            last_exc = e
            _CACHED_FN = None
    raise last_exc


# revision 5
# speedup vs baseline: 1.4627x; 1.0075x over previous
"""Trainium2 Bass kernel for Transformer-XL style relative-position attention.

Problem: B=1, L=2048, D=1024, H=16 heads, dh=64. 8 NeuronCores.
Sharding: heads across cores (2 heads/core), QKV column-parallel,
output projection row-parallel.

I/O strategy (dispatch-overhead optimized):
  * ONE packed bf16 input blob per core (~3.4 MB): the core's 128-row
    D-slice of qT/kT/vT/posT, its head-sliced weights, and biases. One
    copy of every tensor is shipped in total across the 8 cores.
  * On device, the activation slices are AllGather'd (HBM collective)
    so every core sees the full qT/kT/vT/posT.
  * The 8 partial [L, D] f32 outputs from the row-parallel output
    projection are ReduceScatter'd so each core returns only its
    [L/8, D] f32 slice; the host just concatenates.

Per-core device program (scores computed TRANSPOSED, S^T[j, l]):
  1. Projections: qT/kT ([dout, L], lhsT=W^T slices, rhs=x^T), v ([L, dout]).
     Two q variants: q1 = scale*(q + bq + r_w_bias), q2 = scale*(q + bq + r_r_bias).
  2. pe^T[h] = r_kernel[h]^T @ pos_enc^T  ([dh, P]); cols beyond P zero-padded.
  3. rel[l, p] = q2_l . pe_p computed per l-tile, written to DRAM scratch SK with a
     *skewed* DRAM access pattern so SK[l, 128 + j] = rel[l, 2048 - l + j]
     (the _rel_shift). Read back with DMA-transpose (XBAR) as [j, l] tiles.
  4. S^T tile = kT-tile^T @ q1-chunk (+ rel via DVE add), P^T = exp(S^T) (ACT),
     diagonal blocks masked by an upper-triangular 0/1 mask after exp.
  5. AV: psum[l, 0:65] += P^T-subtile^T @ [v | 1]; col 64 = softmax denominator.
     Normalize with reciprocal * tensor_scalar.
  6. Output projection: attn tiles transposed via PE, matmul with Wo slice,
     + bo (bo packed only into core 0's blob), partials to DRAM f32.
  7. ReduceScatter(add) partials -> [L/8, D] f32 -> output.
"""
import sys

for p in ('/opt/trn_rl_repo', '/root/.axon_site/_ro/trn_rl_repo'):
    if p not in sys.path:
        sys.path.insert(0, p)

import numpy as np
import ml_dtypes

import bass_rust
import concourse.bass as bass
import concourse.mybir as mybir
import concourse.tile as tile
from concourse.masks import make_identity, make_upper_triangular

BF16 = mybir.dt.bfloat16
F32 = mybir.dt.float32
NPBF16 = ml_dtypes.bfloat16

L = 2048
D = 1024
H = 16
DH = 64
NCORES = 8
HPC = H // NCORES          # heads per core = 2
DLOC = HPC * DH            # per-core dout slice = 128
LLOC = L // NCORES         # per-core output rows = 256
P_POS = L + 1              # 2049
PE_W = 2176                # pe cols incl 127 zero-pad (covers masked diag region)
SKW = 2304                 # SK scratch row width: 128 left margin + 2048 + margin
SCALE = DH ** -0.5
NT = L // 128              # 16 l-tiles
NCH = L // 512             # 4 l-chunks

# packed blob layout (bf16 elements)
ACT_W = 8208               # 3*2048 + 2049 pos + 15 pad
ACT_E = 128 * ACT_W        # 1050624
W_E = D * DLOC             # 131072
WQOFF = ACT_E
WKOFF = WQOFF + W_E
WVOFF = WKOFF + W_E
RKOFF = WVOFF + W_E
WOOFF = RKOFF + W_E
BQ1OFF = WOOFF + W_E
BQ2OFF = BQ1OFF + DLOC
BKBOFF = BQ2OFF + DLOC
BVBOFF = BKBOFF + DLOC
BOOFF = BVBOFF + DLOC
TOT_E = BOOFF + D          # 1707520

# ---------------------------------------------------------------------------
# Tile/walrus compatibility patches (this walrus build accepts at most ONE
# sync wait per instruction; Tile can emit more). Hoist extras onto standalone
# EventSemaphore instructions, and split the kernel-tail drain's waits.
# ---------------------------------------------------------------------------
_PATCHED = False


def _apply_tile_patches():
    global _PATCHED
    if _PATCHED:
        return
    _PATCHED = True

    def _drain_and_barrier(self, tick_clock, wait_clock):
        nc = self.nc
        probe = mybir.InstNoOp(
            name="drain_wait_probe", ins=[], outs=[], engine=mybir.EngineType.SP
        )
        wait_clock.add_sem_waits(
            probe, bass_rust.ScopedClock({None: tick_clock.global_clock})
        )
        si = probe.sync_info
        waits = list(si.on_wait) if si is not None else []
        sems_by_name = {s.name: s for s in self.sems.allocated().values()}
        for w in waits:
            sem = sems_by_name.get(w.ant_name)
            assert sem is not None and w.wait_mode == "sem-ge-imm"
            nc.sync.wait_ge(sem, w.wait_value)
        nc.sync.drain()
        nc.all_engine_barrier()
        popped = nc._tile_sem_poison_stack.pop()
        assert popped is self._sem_poison
        nc.clear_and_free_semaphores(list(self.sems.allocated().values()))
        nc.all_engine_barrier()

    _orig_add = tile.TileContext._add_instruction
    ctr = [0]

    def _add_instruction(self, inst):
        si = inst.sync_info
        waits = list(si.on_wait) if si is not None else []
        if len(waits) > 1:
            best, order = {}, []
            for w in waits:
                k = w.ant_name
                if k not in best:
                    order.append(k)
                    best[k] = w
                elif (w.wait_value or 0) > (best[k].wait_value or 0):
                    best[k] = w
            waits = [best[k] for k in order]
            for w in waits[:-1]:
                ctr[0] += 1
                ev = mybir.InstEventSemaphore(
                    name=f"{inst.name}_hoistw{ctr[0]}",
                    ins=[],
                    outs=[],
                    engine=inst.engine,
                    sync_info=bass_rust.SyncInfo(on_wait=[w], on_update=[]),
                )
                _orig_add(self, ev)
            inst.sync_info = bass_rust.SyncInfo(
                on_wait=[waits[-1]], on_update=list(si.on_update)
            )
        _orig_add(self, inst)

    tile.TileContext._drain_and_barrier = _drain_and_barrier
    tile.TileContext._add_instruction = _add_instruction


# ---------------------------------------------------------------------------
# Device program
# ---------------------------------------------------------------------------
_CACHED_NC = None


def build_program():
    global _CACHED_NC
    if _CACHED_NC is not None:
        return _CACHED_NC
    _apply_tile_patches()

    nc = bass.Bass()
    blob = nc.dram_tensor("blob", [TOT_E], BF16, kind="ExternalInput")
    outp = nc.dram_tensor("outp", [LLOC, D], F32, kind="ExternalOutput")

    ND = D // 128  # 8 din tiles
    Exp = mybir.ActivationFunctionType.Exp
    Copy = mybir.ActivationFunctionType.Copy
    Ident = mybir.ActivationFunctionType.Identity
    ADD = mybir.AluOpType.add
    MULT = mybir.AluOpType.mult
    GROUP = [list(range(NCORES))]

    with tile.TileContext(nc) as tc:
        with (
            tc.tile_pool(name="constp", bufs=1) as constp,
            tc.tile_pool(name="acts", bufs=1) as acts,
            tc.tile_pool(name="vsp", bufs=1) as vsp,
            tc.tile_pool(name="ps", bufs=1, space="PSUM") as ps,
            tc.tile_pool(name="dramp", bufs=1, space="DRAM") as dramp,
        ):
            # ---- AllGather the activation slices ----
            agin = dramp.tile([128, ACT_W], BF16, name="agin")
            agout = dramp.tile([NCORES * 128, ACT_W], BF16, name="agout",
                               addr_space="Shared")
            nc.gpsimd.dma_start(
                out=agin[:, :],
                in_=bass.AP(blob, 0, [[ACT_W, 128], [1, ACT_W]]),
            )
            nc.gpsimd.collective_compute(
                "AllGather", mybir.AluOpType.bypass,
                replica_groups=GROUP,
                ins=[agin.opt()], outs=[agout.opt()],
            )

            # ---- constants (weights/biases from the blob) ----
            def load_w_tiles(off, name):
                ts = []
                for d in range(ND):
                    t = constp.tile([128, DLOC], BF16, name=f"{name}{d}")
                    nc.sync.dma_start(
                        out=t,
                        in_=bass.AP(blob, off + 128 * DLOC * d,
                                    [[DLOC, 128], [1, DLOC]]),
                    )
                    ts.append(t)
                return ts

            wq_t = load_w_tiles(WQOFF, "wq_t")
            wk_t = load_w_tiles(WKOFF, "wk_t")
            wv_t = load_w_tiles(WVOFF, "wv_t")
            rk_t = load_w_tiles(RKOFF, "rk_t")
            wo_h = []
            for h in range(HPC):
                t = constp.tile([DH, D], BF16, name=f"wo_h{h}")
                nc.sync.dma_start(
                    out=t,
                    in_=bass.AP(blob, WOOFF + DH * D * h, [[D, DH], [1, D]]),
                )
                wo_h.append(t)

            def load_bias(off, name):
                t = constp.tile([DLOC, 1], F32, name=name)
                nc.gpsimd.dma_start(
                    out=t, in_=bass.AP(blob, off, [[1, DLOC], [1, 1]])
                )
                return t

            bq1_t = load_bias(BQ1OFF, "bq1_t")
            bq2_t = load_bias(BQ2OFF, "bq2_t")
            bkb_t = load_bias(BKBOFF, "bkb_t")
            bvb_t = load_bias(BVBOFF, "bvb_t")

            # bo broadcast to [128, D] via PE (ones[1,128]^T outer bo[1,D])
            bo_row = constp.tile([1, D], BF16, name="bo_row")
            nc.sync.dma_start(
                out=bo_row, in_=bass.AP(blob, BOOFF, [[D, 1], [1, D]])
            )
            ones_c = constp.tile([1, 128], BF16, name="ones_c")
            nc.vector.memset(ones_c, 1.0)
            bo_full = constp.tile([128, D], F32, name="bo_full")
            for half in range(2):
                hs = slice(512 * half, 512 * (half + 1))
                pb = ps.tile([128, 512], F32, tag="cont", bufs=3, name="pb")
                nc.tensor.matmul(pb, ones_c, bo_row[:, hs],
                                 start=True, stop=True)
                nc.scalar.activation(bo_full[:, hs], pb, Copy)

            umask = constp.tile([128, 128], BF16, name="umask")
            make_upper_triangular(nc, umask, val=1.0)
            ident = constp.tile([128, 128], BF16, name="ident")
            make_identity(nc, ident)
            ident32 = constp.tile([128, 128], F32, name="ident32")
            make_identity(nc, ident32)

            # ---- persistent activations ----
            q1 = acts.tile([DLOC, L], BF16, name="q1")
            q2 = acts.tile([DLOC, L], BF16, name="q2")
            k1 = acts.tile([DLOC, L], BF16, name="k1")
            vpT = acts.tile([DLOC, L], BF16, name="vpT")
            peT = acts.tile([128, PE_W], BF16, name="peT")
            aT = [acts.tile([DH, L], BF16, name=f"aT{h}") for h in range(HPC)]
            recip_all = [
                acts.tile([128, NT], F32, name=f"recip{h}") for h in range(HPC)
            ]
            vS = [vsp.tile([128, 130], BF16, name=f"vS{j}") for j in range(NT)]
            sk = [
                dramp.tile([L, SKW], BF16, name=f"sk{h}") for h in range(HPC)
            ]
            rsin = dramp.tile([L, D], F32, name="rsin")
            rsout = dramp.tile([LLOC, D], F32, name="rsout")

            # ================= stage 1: projections =================
            with tc.tile_pool(name="inp", bufs=1) as inp:
                def load_in_tiles(col0, name, cols):
                    ts = []
                    for d in range(ND):
                        t = inp.tile([128, cols], BF16, name=f"{name}{d}")
                        eng = nc.sync if d % 2 == 0 else nc.scalar
                        eng.dma_start(
                            out=t,
                            in_=agout[128 * d:128 * (d + 1),
                                      col0:col0 + cols],
                        )
                        ts.append(t)
                    return ts

                qT_s = load_in_tiles(0, "qT_s", L)
                kT_s = load_in_tiles(L, "kT_s", L)
                vT_s = load_in_tiles(2 * L, "vT_s", L)
                posT_s = load_in_tiles(3 * L, "posT_s", P_POS)

                # projections grouped by tensor, matching DMA arrival order
                for c in range(NCH):
                    sl = slice(512 * c, 512 * (c + 1))
                    pq = ps.tile([128, 512], F32, tag="cont", bufs=3, name="pq")
                    for d in range(ND):
                        nc.tensor.matmul(
                            pq, wq_t[d], qT_s[d][:, sl],
                            start=(d == 0), stop=(d == ND - 1),
                        )
                    nc.scalar.activation(q1[:, sl], pq, Ident,
                                         bias=bq1_t, scale=SCALE)
                    nc.scalar.activation(q2[:, sl], pq, Ident,
                                         bias=bq2_t, scale=SCALE)
                for c in range(NCH):
                    sl = slice(512 * c, 512 * (c + 1))
                    pk = ps.tile([128, 512], F32, tag="cont", bufs=3, name="pk")
                    for d in range(ND):
                        nc.tensor.matmul(
                            pk, wk_t[d], kT_s[d][:, sl],
                            start=(d == 0), stop=(d == ND - 1),
                        )
                    nc.scalar.activation(k1[:, sl], pk, Ident, bias=bkb_t)
                for c in range(NCH):
                    sl = slice(512 * c, 512 * (c + 1))
                    pv = ps.tile([128, 512], F32, tag="cont", bufs=3, name="pv")
                    for d in range(ND):
                        nc.tensor.matmul(
                            pv, wv_t[d], vT_s[d][:, sl],
                            start=(d == 0), stop=(d == ND - 1),
                        )
                    nc.scalar.activation(vpT[:, sl], pv, Ident, bias=bvb_t)

                # pe^T (both heads stacked): rows 64h..64h+64 = head h
                pe_chunks = [(0, 512), (512, 512), (1024, 512), (1536, 512),
                             (2048, 1)]
                for (cs, cw) in pe_chunks:
                    pp = ps.tile([128, 512], F32, tag="cont", bufs=3,
                                 name="pp")
                    for d in range(ND):
                        nc.tensor.matmul(
                            pp[:, 0:cw], rk_t[d], posT_s[d][:, cs:cs + cw],
                            start=(d == 0), stop=(d == ND - 1),
                        )
                    nc.scalar.activation(peT[:, cs:cs + cw], pp[:, 0:cw], Copy)
                nc.vector.memset(peT[:, P_POS:PE_W], 0.0)

            # v transposes -> vS[t] = [v_h0 | 1 | v_h1 | 1]
            for t in range(NT):
                pvt = ps.tile([128, 128], BF16, tag="mm128", bufs=1,
                              name="pvt")
                nc.tensor.transpose(pvt, vpT[:, 128 * t:128 * (t + 1)], ident)
                nc.scalar.activation(vS[t][:, 0:DH], pvt[:, 0:DH], Copy)
                nc.scalar.activation(vS[t][:, 65:65 + DH], pvt[:, DH:DLOC],
                                     Copy)
                nc.vector.memset(vS[t][:, 64:65], 1.0)
                nc.vector.memset(vS[t][:, 129:130], 1.0)

            work = exit_stack_work = tc.tile_pool(name="work", bufs=1)
            work = work.__enter__()

            # ================= stage 2: rel -> skewed DRAM =================
            for t in range(NT):
                for h in range(HPC):
                    hsl = slice(DH * h, DH * (h + 1))
                    l0 = 128 * t
                    pmin = 1921 - l0
                    wrel = PE_W - pmin  # 128*t + 255
                    rel_sb = work.tile([128, PE_W], BF16, tag="rel_sb",
                                       bufs=3, name="rel_sb")
                    cs = 0
                    while cs < wrel:
                        cw = min(512, wrel - cs)
                        pr = ps.tile([128, 512], F32, tag="relp", bufs=2,
                                     name="pr")
                        nc.tensor.matmul(
                            pr[:, 0:cw], q2[hsl, l0:l0 + 128],
                            peT[hsl, pmin + cs:pmin + cs + cw],
                            start=True, stop=True,
                        )
                        nc.scalar.activation(
                            rel_sb[:, cs:cs + cw], pr[:, 0:cw], Copy
                        )
                        cs += cw
                    dst = bass.AP(
                        sk[h].tensor,
                        l0 * (SKW + 1) + pmin - 1920,
                        [[SKW + 1, 128], [1, wrel]],
                    )
                    nc.gpsimd.dma_start(out=dst, in_=rel_sb[:, 0:wrel])

            # ================= stage 3: scores/softmax/AV =================
            for h in range(HPC):
                hsl = slice(DH * h, DH * (h + 1))
                for c in range(NCH):
                    lc = 512 * c
                    nJ = 4 * (c + 1)
                    avp = ps.tile([65, 512], F32, tag="avT", bufs=2,
                                  name="avp")
                    pTs = []

                    def emit_av(J):
                        nc.tensor.matmul(
                            avp, vS[J][:, 65 * h:65 * (h + 1)], pTs[J],
                            start=(J == 0), stop=(J == nJ - 1),
                        )

                    for J in range(nJ):
                        j0 = 128 * J
                        col0 = max(0, j0 - lc)
                        wv_ = 512 - col0
                        pS = ps.tile([128, 512], F32, tag="cont", bufs=3,
                                     name="pS")
                        nc.tensor.matmul(
                            pS[:, 0:wv_], k1[hsl, j0:j0 + 128],
                            q1[hsl, lc + col0:lc + 512],
                            start=True, stop=True,
                        )
                        relT = work.tile([128, 512], BF16, tag="relT", bufs=6,
                                         name="relT")
                        nc.scalar.dma_start(
                            out=relT[:, 0:wv_],
                            in_=sk[h][lc + col0:lc + 512, 128 + j0:256 + j0],
                            transpose=True,
                        )
                        sc = work.tile([128, 512], F32, tag="sc", bufs=4,
                                       name="sc")
                        nc.vector.tensor_tensor(
                            sc[:, 0:wv_], pS[:, 0:wv_], relT[:, 0:wv_], ADD
                        )
                        pT = work.tile([128, 512], BF16, tag="pT", bufs=8,
                                       name="pT")
                        nc.scalar.activation(pT[:, col0:512], sc[:, 0:wv_],
                                             Exp)
                        if col0 > 0:
                            nc.gpsimd.memset(pT[:, 0:col0], 0.0)
                        if J >= 4 * c:
                            nc.gpsimd.tensor_tensor(
                                pT[:, col0:col0 + 128],
                                pT[:, col0:col0 + 128], umask, MULT,
                            )
                        pTs.append(pT)
                        emit_av(J)

                    # evict: rows 0..63 -> aT (bf16); denom row 64 -> f32
                    nc.scalar.activation(
                        aT[h][:, lc:lc + 512], avp[0:DH, :], Copy
                    )
                    den = work.tile([1, 512], F32, tag="den", bufs=1,
                                    name="den")
                    nc.scalar.activation(den, avp[DH:DH + 1, :], Copy)
                    pd = ps.tile([128, 4], F32, tag="mm128", bufs=1,
                                 name="pd")
                    for s in range(4):
                        nc.tensor.transpose(
                            pd[:, s:s + 1], den[:, 128 * s:128 * (s + 1)],
                            ident32[0:1, 0:1]
                        )
                    nc.vector.reciprocal(
                        recip_all[h][:, 4 * c:4 * c + 4], pd
                    )

            # ================= stage 4: output projection =================
            for t in range(NT):
                tsl = slice(128 * t, 128 * (t + 1))
                out_sb = work.tile([128, D], F32, tag="out_sb", bufs=2,
                                   name="out_sb")
                for oc in range(2):
                    osl = slice(512 * oc, 512 * (oc + 1))
                    po0 = ps.tile([128, 512], F32, tag="cont", bufs=3,
                                  name="po0")
                    nc.tensor.matmul(po0, aT[0][:, tsl], wo_h[0][:, osl],
                                     start=True, stop=True)
                    nc.vector.scalar_tensor_tensor(
                        out_sb[:, osl], po0, recip_all[0][:, t:t + 1],
                        bo_full[:, osl], MULT, ADD,
                    )
                    po1 = ps.tile([128, 512], F32, tag="cont", bufs=3,
                                  name="po1")
                    nc.tensor.matmul(po1, aT[1][:, tsl], wo_h[1][:, osl],
                                     start=True, stop=True)
                    nc.vector.scalar_tensor_tensor(
                        out_sb[:, osl], po1, recip_all[1][:, t:t + 1],
                        out_sb[:, osl], MULT, ADD,
                    )
                nc.sync.dma_start(out=rsin[tsl, :], in_=out_sb)

            # ============ stage 5: ReduceScatter -> output slice ============
            nc.gpsimd.collective_compute(
                "ReduceScatter", ADD,
                replica_groups=GROUP,
                ins=[rsin.opt()], outs=[rsout.opt()],
            )
            nc.sync.dma_start(out=outp[:, :], in_=rsout[:])

            exit_stack_work.__exit__(None, None, None)

    _CACHED_NC = nc
    return nc


# ---------------------------------------------------------------------------
# Host wrapper
# ---------------------------------------------------------------------------
def _prep_inputs(q, k, v, pos_enc, Wq, bq, Wk, bk, Wv, bv, Wo, bo,
                 r_w_bias, r_r_bias, r_kernel):
    q2d = np.asarray(q, np.float32).reshape(L, D)
    k2d = np.asarray(k, np.float32).reshape(L, D)
    v2d = np.asarray(v, np.float32).reshape(L, D)
    p2d = np.asarray(pos_enc, np.float32)
    rwb = np.asarray(r_w_bias, np.float32).reshape(H, DH)
    rrb = np.asarray(r_r_bias, np.float32).reshape(H, DH)
    Wq = np.asarray(Wq, np.float32)
    Wk = np.asarray(Wk, np.float32)
    Wv = np.asarray(Wv, np.float32)
    Wo = np.asarray(Wo, np.float32)
    rkn = np.asarray(r_kernel, np.float32)
    bq = np.asarray(bq, np.float32)
    bk = np.asarray(bk, np.float32)
    bv = np.asarray(bv, np.float32)
    bo = np.asarray(bo, np.float32)

    in_maps = []
    for c in range(NCORES):
        sl = slice(DLOC * c, DLOC * (c + 1))
        hsl = slice(HPC * c, HPC * (c + 1))
        blob = np.zeros(TOT_E, NPBF16)
        act = blob[:ACT_E].reshape(128, ACT_W)
        act[:, 0:L] = q2d[:, sl].T
        act[:, L:2 * L] = k2d[:, sl].T
        act[:, 2 * L:3 * L] = v2d[:, sl].T
        act[:, 3 * L:3 * L + P_POS] = p2d[:, sl].T
        blob[WQOFF:WQOFF + W_E] = Wq[sl].T.astype(NPBF16).ravel()
        blob[WKOFF:WKOFF + W_E] = Wk[sl].T.astype(NPBF16).ravel()
        blob[WVOFF:WVOFF + W_E] = Wv[sl].T.astype(NPBF16).ravel()
        rk_c = rkn[hsl]  # [2, D, DH]
        blob[RKOFF:RKOFF + W_E] = np.concatenate(
            [rk_c[0], rk_c[1]], axis=1).astype(NPBF16).ravel()
        blob[WOOFF:WOOFF + W_E] = Wo[:, sl].T.astype(NPBF16).ravel()
        bq_c = bq[sl]
        blob[BQ1OFF:BQ1OFF + DLOC] = (
            SCALE * (bq_c + rwb[hsl].reshape(DLOC))).astype(NPBF16)
        blob[BQ2OFF:BQ2OFF + DLOC] = (
            SCALE * (bq_c + rrb[hsl].reshape(DLOC))).astype(NPBF16)
        blob[BKBOFF:BKBOFF + DLOC] = bk[sl].astype(NPBF16)
        blob[BVBOFF:BVBOFF + DLOC] = bv[sl].astype(NPBF16)
        if c == 0:
            blob[BOOFF:BOOFF + D] = bo.astype(NPBF16)
        in_maps.append({"blob": blob})
    return in_maps


# ---------------------------------------------------------------------------
# Cached sharded PJRT executable (built once per process)
# ---------------------------------------------------------------------------
_CACHED_FN = None


def _get_fn():
    global _CACHED_FN
    if _CACHED_FN is not None:
        return _CACHED_FN
    import jax
    from jax.sharding import Mesh, PartitionSpec
    from jax.experimental.shard_map import shard_map
    from concourse import bass2jax

    nc = build_program()
    bass2jax.install_neuronx_cc_hook()
    partition_name = (
        nc.partition_id_tensor.name if nc.partition_id_tensor else None
    )
    in_names, out_names, out_avals, zero_shapes = [], [], [], []
    for alloc in nc.m.functions[0].allocations:
        if not isinstance(alloc, mybir.MemoryLocationSet):
            continue
        name = alloc.memorylocations[0].name
        if alloc.kind == "ExternalInput":
            if name != partition_name:
                in_names.append(name)
        elif alloc.kind == "ExternalOutput":
            shape = tuple(alloc.tensor_shape)
            dtype = mybir.dt.np(alloc.dtype)
            out_names.append(name)
            out_avals.append(jax.core.ShapedArray(shape, dtype))
            zero_shapes.append((shape, dtype))
    n_params = len(in_names)
    n_outs = len(out_avals)
    all_in_names = list(in_names) + list(out_names)
    if partition_name is not None:
        all_in_names.append(partition_name)

    def _body(*args):
        operands = list(args)
        if partition_name is not None:
            operands.append(bass2jax.partition_id_tensor())
        outs = bass2jax._bass_exec_p.bind(
            *operands,
            out_avals=tuple(out_avals),
            in_names=tuple(all_in_names),
            out_names=tuple(out_names),
            lowering_input_output_aliases=(),
            sim_require_finite=True,
            sim_require_nnan=True,
            nc=nc,
        )
        return tuple(outs)

    donate = tuple(range(n_params, n_params + n_outs))
    devices = jax.devices()[:NCORES]
    mesh = Mesh(np.asarray(devices), ("core",))
    in_specs = (PartitionSpec("core"),) * (n_params + n_outs)
    out_specs = (PartitionSpec("core"),) * n_outs
    fn = jax.jit(
        shard_map(_body, mesh=mesh, in_specs=in_specs,
                  out_specs=out_specs, check_rep=False),
        donate_argnums=donate,
        keep_unused=True,
    )
    # pre-sharding args with this avoids an in-call reshard (~2-5 ms)
    from jax.sharding import NamedSharding
    sharding = NamedSharding(mesh, PartitionSpec("core"))
    _CACHED_FN = (fn, in_names, out_names, zero_shapes, sharding)
    return _CACHED_FN


def kernel(**inputs):
    global _CACHED_FN
    in_maps = _prep_inputs(**inputs)
    last_exc = None
    for attempt in range(2):
        try:
            import jax
            fn, in_names, out_names, zero_shapes, sharding = _get_fn()
            concat_in = [
                jax.device_put(
                    np.concatenate([m[name] for m in in_maps], axis=0),
                    sharding)
                for name in in_names
            ]
            zeros = [
                jax.device_put(np.zeros((NCORES * s[0], *s[1:]), dt),
                               sharding)
                for (s, dt) in zero_shapes
            ]
            outs = fn(*concat_in, *zeros)
            out = np.asarray(outs[out_names.index("outp")])  # [L, D] f32
            return out.reshape(1, L, D).astype(np.float32)
        except Exception as e:  # transient axon/mesh hiccup: rebuild once
            last_exc = e
            _CACHED_FN = None
    raise last_exc


# revision 8
# speedup vs baseline: 1.7511x; 1.1972x over previous
"""Trainium2 Bass kernel for Transformer-XL style relative-position attention.

Problem: B=1, L=2048, D=1024, H=16 heads, dh=64. 8 NeuronCores.
Sharding: heads across cores (2 heads/core), QKV column-parallel,
output projection row-parallel.

I/O strategy (dispatch-overhead optimized):
  * ONE packed bf16 input blob per core (~3.4 MB): the core's 128-row
    D-slice of qT/kT/vT/posT, its head-sliced weights, and biases. One
    copy of every tensor is shipped in total across the 8 cores.
  * On device, the activation slices are AllGather'd (HBM collective)
    so every core sees the full qT/kT/vT/posT.
  * The 8 partial [L, D] f32 outputs from the row-parallel output
    projection are ReduceScatter'd so each core returns only its
    [L/8, D] f32 slice; the host just concatenates.

Per-core device program (scores computed TRANSPOSED, S^T[j, l]):
  1. Projections: qT/kT ([dout, L], lhsT=W^T slices, rhs=x^T), v ([L, dout]).
     Two q variants: q1 = scale*(q + bq + r_w_bias), q2 = scale*(q + bq + r_r_bias).
  2. pe^T[h] = r_kernel[h]^T @ pos_enc^T  ([dh, P]); cols beyond P zero-padded.
  3. rel[l, p] = q2_l . pe_p computed per l-tile, written to DRAM scratch SK with a
     *skewed* DRAM access pattern so SK[l, 128 + j] = rel[l, 2048 - l + j]
     (the _rel_shift). Read back with DMA-transpose (XBAR) as [j, l] tiles.
  4. S^T tile = kT-tile^T @ q1-chunk (+ rel via DVE add), P^T = exp(S^T) (ACT),
     diagonal blocks masked by an upper-triangular 0/1 mask after exp.
  5. AV: psum[l, 0:65] += P^T-subtile^T @ [v | 1]; col 64 = softmax denominator.
     Normalize with reciprocal * tensor_scalar.
  6. Output projection: attn tiles transposed via PE, matmul with Wo slice,
     + bo (bo packed only into core 0's blob), partials to DRAM f32.
  7. ReduceScatter(add) partials -> [L/8, D] f32 -> output.
"""
import sys

for p in ('/opt/trn_rl_repo', '/root/.axon_site/_ro/trn_rl_repo'):
    if p not in sys.path:
        sys.path.insert(0, p)

import numpy as np
import ml_dtypes

import bass_rust
import concourse.bass as bass
import concourse.mybir as mybir
import concourse.tile as tile
from concourse.masks import make_identity, make_upper_triangular

BF16 = mybir.dt.bfloat16
F32 = mybir.dt.float32
NPBF16 = ml_dtypes.bfloat16

L = 2048
D = 1024
H = 16
DH = 64
NCORES = 8
HPC = H // NCORES          # heads per core = 2
DLOC = HPC * DH            # per-core dout slice = 128
LLOC = L // NCORES         # per-core output rows = 256
P_POS = L + 1              # 2049
PE_W = 2176                # pe cols incl 127 zero-pad (covers masked diag region)
SKW = 2304                 # SK scratch row width: 128 left margin + 2048 + margin
SCALE = DH ** -0.5
NT = L // 128              # 16 l-tiles
NCH = L // 512             # 4 l-chunks

# packed blob layout (bf16 elements)
ACT_W = 8208               # 3*2048 + 2049 pos + 15 pad
ACT_E = 128 * ACT_W        # 1050624
W_E = D * DLOC             # 131072
WQOFF = ACT_E
WKOFF = WQOFF + W_E
WVOFF = WKOFF + W_E
RKOFF = WVOFF + W_E
WOOFF = RKOFF + W_E
BQ1OFF = WOOFF + W_E
BQ2OFF = BQ1OFF + DLOC
BKBOFF = BQ2OFF + DLOC
BVBOFF = BKBOFF + DLOC
BOOFF = BVBOFF + DLOC
TOT_E = BOOFF + D          # 1707520

# ---------------------------------------------------------------------------
# Tile/walrus compatibility patches (this walrus build accepts at most ONE
# sync wait per instruction; Tile can emit more). Hoist extras onto standalone
# EventSemaphore instructions, and split the kernel-tail drain's waits.
# ---------------------------------------------------------------------------
_PATCHED = False


def _apply_tile_patches():
    global _PATCHED
    if _PATCHED:
        return
    _PATCHED = True

    def _drain_and_barrier(self, tick_clock, wait_clock):
        nc = self.nc
        probe = mybir.InstNoOp(
            name="drain_wait_probe", ins=[], outs=[], engine=mybir.EngineType.SP
        )
        wait_clock.add_sem_waits(
            probe, bass_rust.ScopedClock({None: tick_clock.global_clock})
        )
        si = probe.sync_info
        waits = list(si.on_wait) if si is not None else []
        sems_by_name = {s.name: s for s in self.sems.allocated().values()}
        for w in waits:
            sem = sems_by_name.get(w.ant_name)
            assert sem is not None and w.wait_mode == "sem-ge-imm"
            nc.sync.wait_ge(sem, w.wait_value)
        nc.sync.drain()
        nc.all_engine_barrier()
        popped = nc._tile_sem_poison_stack.pop()
        assert popped is self._sem_poison
        nc.clear_and_free_semaphores(list(self.sems.allocated().values()))
        nc.all_engine_barrier()

    _orig_add = tile.TileContext._add_instruction
    ctr = [0]

    def _add_instruction(self, inst):
        si = inst.sync_info
        waits = list(si.on_wait) if si is not None else []
        if len(waits) > 1:
            best, order = {}, []
            for w in waits:
                k = w.ant_name
                if k not in best:
                    order.append(k)
                    best[k] = w
                elif (w.wait_value or 0) > (best[k].wait_value or 0):
                    best[k] = w
            waits = [best[k] for k in order]
            for w in waits[:-1]:
                ctr[0] += 1
                ev = mybir.InstEventSemaphore(
                    name=f"{inst.name}_hoistw{ctr[0]}",
                    ins=[],
                    outs=[],
                    engine=inst.engine,
                    sync_info=bass_rust.SyncInfo(on_wait=[w], on_update=[]),
                )
                _orig_add(self, ev)
            inst.sync_info = bass_rust.SyncInfo(
                on_wait=[waits[-1]], on_update=list(si.on_update)
            )
        _orig_add(self, inst)

    tile.TileContext._drain_and_barrier = _drain_and_barrier
    tile.TileContext._add_instruction = _add_instruction


# ---------------------------------------------------------------------------
# Device program
# ---------------------------------------------------------------------------
_CACHED_NC = None


def build_program():
    global _CACHED_NC
    if _CACHED_NC is not None:
        return _CACHED_NC
    _apply_tile_patches()

    nc = bass.Bass()
    blob = nc.dram_tensor("blob", [TOT_E], BF16, kind="ExternalInput")
    outp = nc.dram_tensor("outp", [LLOC, D], F32, kind="ExternalOutput")

    ND = D // 128  # 8 din tiles
    Exp = mybir.ActivationFunctionType.Exp
    Copy = mybir.ActivationFunctionType.Copy
    Ident = mybir.ActivationFunctionType.Identity
    ADD = mybir.AluOpType.add
    MULT = mybir.AluOpType.mult
    GROUP = [list(range(NCORES))]

    with tile.TileContext(nc) as tc:
        with (
            tc.tile_pool(name="constp", bufs=1) as constp,
            tc.tile_pool(name="acts", bufs=1) as acts,
            tc.tile_pool(name="vsp", bufs=1) as vsp,
            tc.tile_pool(name="ps", bufs=1, space="PSUM") as ps,
            tc.tile_pool(name="dramp", bufs=1, space="DRAM") as dramp,
        ):
            # ---- AllGather the activation slices ----
            agin = dramp.tile([128, ACT_W], BF16, name="agin")
            agout = dramp.tile([NCORES * 128, ACT_W], BF16, name="agout",
                               addr_space="Shared")
            nc.gpsimd.dma_start(
                out=agin[:, :],
                in_=bass.AP(blob, 0, [[ACT_W, 128], [1, ACT_W]]),
            )
            nc.gpsimd.collective_compute(
                "AllGather", mybir.AluOpType.bypass,
                replica_groups=GROUP,
                ins=[agin.opt()], outs=[agout.opt()],
            )

            # ---- constants (weights/biases from the blob) ----
            def load_w_tiles(off, name):
                ts = []
                for d in range(ND):
                    t = constp.tile([128, DLOC], BF16, name=f"{name}{d}")
                    nc.sync.dma_start(
                        out=t,
                        in_=bass.AP(blob, off + 128 * DLOC * d,
                                    [[DLOC, 128], [1, DLOC]]),
                    )
                    ts.append(t)
                return ts

            wq_t = load_w_tiles(WQOFF, "wq_t")
            wk_t = load_w_tiles(WKOFF, "wk_t")
            wv_t = load_w_tiles(WVOFF, "wv_t")
            rk_t = load_w_tiles(RKOFF, "rk_t")
            wo_h = []
            for h in range(HPC):
                t = constp.tile([DH, D], BF16, name=f"wo_h{h}")
                nc.sync.dma_start(
                    out=t,
                    in_=bass.AP(blob, WOOFF + DH * D * h, [[D, DH], [1, D]]),
                )
                wo_h.append(t)

            def load_bias(off, name):
                t = constp.tile([DLOC, 1], F32, name=name)
                nc.gpsimd.dma_start(
                    out=t, in_=bass.AP(blob, off, [[1, DLOC], [1, 1]])
                )
                return t

            bq1_t = load_bias(BQ1OFF, "bq1_t")
            bq2_t = load_bias(BQ2OFF, "bq2_t")
            bkb_t = load_bias(BKBOFF, "bkb_t")
            bvb_t = load_bias(BVBOFF, "bvb_t")

            # bo broadcast to [128, D] via PE (ones[1,128]^T outer bo[1,D])
            bo_row = constp.tile([1, D], BF16, name="bo_row")
            nc.sync.dma_start(
                out=bo_row, in_=bass.AP(blob, BOOFF, [[D, 1], [1, D]])
            )
            ones_c = constp.tile([1, 128], BF16, name="ones_c")
            nc.vector.memset(ones_c, 1.0)
            bo_full = constp.tile([128, D], F32, name="bo_full")
            for half in range(2):
                hs = slice(512 * half, 512 * (half + 1))
                pb = ps.tile([128, 512], F32, tag="cont", bufs=3, name="pb")
                nc.tensor.matmul(pb, ones_c, bo_row[:, hs],
                                 start=True, stop=True)
                nc.scalar.activation(bo_full[:, hs], pb, Copy)

            umask = constp.tile([128, 128], BF16, name="umask")
            make_upper_triangular(nc, umask, val=1.0)
            ident = constp.tile([128, 128], BF16, name="ident")
            make_identity(nc, ident)
            ident32 = constp.tile([128, 128], F32, name="ident32")
            make_identity(nc, ident32)

            # ---- persistent activations ----
            q1 = acts.tile([DLOC, L], BF16, name="q1")
            q2 = acts.tile([DLOC, L], BF16, name="q2")
            k1 = acts.tile([DLOC, L], BF16, name="k1")
            vpT = acts.tile([DLOC, L], BF16, name="vpT")
            peT = acts.tile([128, PE_W], BF16, name="peT")
            aT = [acts.tile([DH, L], BF16, name=f"aT{h}") for h in range(HPC)]
            recip_all = [
                acts.tile([128, NT], F32, name=f"recip{h}") for h in range(HPC)
            ]
            vS = [vsp.tile([128, 130], BF16, name=f"vS{j}") for j in range(NT)]
            sk = [
                dramp.tile([L, SKW], BF16, name=f"sk{h}") for h in range(HPC)
            ]
            rsin = dramp.tile([L, D], F32, name="rsin")
            rsout = dramp.tile([LLOC, D], F32, name="rsout")

            # ================= stage 1: projections =================
            with tc.tile_pool(name="inp", bufs=1) as inp:
                def load_in_tiles(col0, name, cols):
                    ts = []
                    for d in range(ND):
                        t = inp.tile([128, cols], BF16, name=f"{name}{d}")
                        eng = nc.sync if d % 2 == 0 else nc.scalar
                        eng.dma_start(
                            out=t,
                            in_=agout[128 * d:128 * (d + 1),
                                      col0:col0 + cols],
                        )
                        ts.append(t)
                    return ts

                qT_s = load_in_tiles(0, "qT_s", L)
                kT_s = load_in_tiles(L, "kT_s", L)
                vT_s = load_in_tiles(2 * L, "vT_s", L)
                posT_s = load_in_tiles(3 * L, "posT_s", P_POS)

                # projections grouped by tensor, matching DMA arrival order
                for c in range(NCH):
                    sl = slice(512 * c, 512 * (c + 1))
                    pq = ps.tile([128, 512], F32, tag="cont", bufs=3, name="pq")
                    for d in range(ND):
                        nc.tensor.matmul(
                            pq, wq_t[d], qT_s[d][:, sl],
                            start=(d == 0), stop=(d == ND - 1),
                        )
                    nc.scalar.activation(q1[:, sl], pq, Ident,
                                         bias=bq1_t, scale=SCALE)
                    nc.scalar.activation(q2[:, sl], pq, Ident,
                                         bias=bq2_t, scale=SCALE)
                for c in range(NCH):
                    sl = slice(512 * c, 512 * (c + 1))
                    pk = ps.tile([128, 512], F32, tag="cont", bufs=3, name="pk")
                    for d in range(ND):
                        nc.tensor.matmul(
                            pk, wk_t[d], kT_s[d][:, sl],
                            start=(d == 0), stop=(d == ND - 1),
                        )
                    nc.scalar.activation(k1[:, sl], pk, Ident, bias=bkb_t)
                for c in range(NCH):
                    sl = slice(512 * c, 512 * (c + 1))
                    pv = ps.tile([128, 512], F32, tag="cont", bufs=3, name="pv")
                    for d in range(ND):
                        nc.tensor.matmul(
                            pv, wv_t[d], vT_s[d][:, sl],
                            start=(d == 0), stop=(d == ND - 1),
                        )
                    nc.scalar.activation(vpT[:, sl], pv, Ident, bias=bvb_t)

                # pe^T (both heads stacked): rows 64h..64h+64 = head h
                pe_chunks = [(0, 512), (512, 512), (1024, 512), (1536, 512),
                             (2048, 1)]
                for (cs, cw) in pe_chunks:
                    pp = ps.tile([128, 512], F32, tag="cont", bufs=3,
                                 name="pp")
                    for d in range(ND):
                        nc.tensor.matmul(
                            pp[:, 0:cw], rk_t[d], posT_s[d][:, cs:cs + cw],
                            start=(d == 0), stop=(d == ND - 1),
                        )
                    nc.scalar.activation(peT[:, cs:cs + cw], pp[:, 0:cw], Copy)
                nc.vector.memset(peT[:, P_POS:PE_W], 0.0)

            # v transposes -> vS[t] = [v_h0 | 1 | v_h1 | 1]
            for t in range(NT):
                pvt = ps.tile([128, 128], BF16, tag="mm128", bufs=1,
                              name="pvt")
                nc.tensor.transpose(pvt, vpT[:, 128 * t:128 * (t + 1)], ident)
                nc.scalar.activation(vS[t][:, 0:DH], pvt[:, 0:DH], Copy)
                nc.scalar.activation(vS[t][:, 65:65 + DH], pvt[:, DH:DLOC],
                                     Copy)
                nc.vector.memset(vS[t][:, 64:65], 1.0)
                nc.vector.memset(vS[t][:, 129:130], 1.0)

            work = exit_stack_work = tc.tile_pool(name="work", bufs=1)
            work = work.__enter__()

            # ================= stage 2: rel -> skewed DRAM =================
            for t in range(NT):
                for h in range(HPC):
                    hsl = slice(DH * h, DH * (h + 1))
                    l0 = 128 * t
                    pmin = 1921 - l0
                    wrel = PE_W - pmin  # 128*t + 255
                    rel_sb = work.tile([128, PE_W], BF16, tag="rel_sb",
                                       bufs=3, name="rel_sb")
                    cs = 0
                    while cs < wrel:
                        cw = min(512, wrel - cs)
                        pr = ps.tile([128, 512], F32, tag="relp", bufs=2,
                                     name="pr")
                        nc.tensor.matmul(
                            pr[:, 0:cw], q2[hsl, l0:l0 + 128],
                            peT[hsl, pmin + cs:pmin + cs + cw],
                            start=True, stop=True,
                        )
                        nc.scalar.activation(
                            rel_sb[:, cs:cs + cw], pr[:, 0:cw], Copy
                        )
                        cs += cw
                    dst = bass.AP(
                        sk[h].tensor,
                        l0 * (SKW + 1) + pmin - 1920,
                        [[SKW + 1, 128], [1, wrel]],
                    )
                    nc.gpsimd.dma_start(out=dst, in_=rel_sb[:, 0:wrel])

            # ================= stage 3: scores/softmax/AV =================
            for h in range(HPC):
                hsl = slice(DH * h, DH * (h + 1))
                for c in range(NCH):
                    lc = 512 * c
                    nJ = 4 * (c + 1)
                    avp = ps.tile([65, 512], F32, tag="avT", bufs=2,
                                  name="avp")
                    pTs = []

                    def emit_av(J):
                        nc.tensor.matmul(
                            avp, vS[J][:, 65 * h:65 * (h + 1)], pTs[J],
                            start=(J == 0), stop=(J == nJ - 1),
                        )

                    for J in range(nJ):
                        j0 = 128 * J
                        col0 = max(0, j0 - lc)
                        wv_ = 512 - col0
                        pS = ps.tile([128, 512], F32, tag="cont", bufs=3,
                                     name="pS")
                        nc.tensor.matmul(
                            pS[:, 0:wv_], k1[hsl, j0:j0 + 128],
                            q1[hsl, lc + col0:lc + 512],
                            start=True, stop=True,
                        )
                        relT = work.tile([128, 512], BF16, tag="relT", bufs=6,
                                         name="relT")
                        nc.scalar.dma_start(
                            out=relT[:, 0:wv_],
                            in_=sk[h][lc + col0:lc + 512, 128 + j0:256 + j0],
                            transpose=True,
                        )
                        sc = work.tile([128, 512], F32, tag="sc", bufs=4,
                                       name="sc")
                        nc.vector.tensor_tensor(
                            sc[:, 0:wv_], pS[:, 0:wv_], relT[:, 0:wv_], ADD
                        )
                        pT = work.tile([128, 512], BF16, tag="pT", bufs=8,
                                       name="pT")
                        nc.scalar.activation(pT[:, col0:512], sc[:, 0:wv_],
                                             Exp)
                        if col0 > 0:
                            nc.gpsimd.memset(pT[:, 0:col0], 0.0)
                        if J >= 4 * c:
                            nc.gpsimd.tensor_tensor(
                                pT[:, col0:col0 + 128],
                                pT[:, col0:col0 + 128], umask, MULT,
                            )
                        pTs.append(pT)
                        emit_av(J)

                    # evict: rows 0..63 -> aT (bf16); denom row 64 -> f32
                    nc.scalar.activation(
                        aT[h][:, lc:lc + 512], avp[0:DH, :], Copy
                    )
                    den = work.tile([1, 512], F32, tag="den", bufs=1,
                                    name="den")
                    nc.scalar.activation(den, avp[DH:DH + 1, :], Copy)
                    pd = ps.tile([128, 4], F32, tag="mm128", bufs=1,
                                 name="pd")
                    for s in range(4):
                        nc.tensor.transpose(
                            pd[:, s:s + 1], den[:, 128 * s:128 * (s + 1)],
                            ident32[0:1, 0:1]
                        )
                    nc.vector.reciprocal(
                        recip_all[h][:, 4 * c:4 * c + 4], pd
                    )

            # ================= stage 4: output projection =================
            for t in range(NT):
                tsl = slice(128 * t, 128 * (t + 1))
                out_sb = work.tile([128, D], F32, tag="out_sb", bufs=2,
                                   name="out_sb")
                for oc in range(2):
                    osl = slice(512 * oc, 512 * (oc + 1))
                    po0 = ps.tile([128, 512], F32, tag="cont", bufs=3,
                                  name="po0")
                    nc.tensor.matmul(po0, aT[0][:, tsl], wo_h[0][:, osl],
                                     start=True, stop=True)
                    nc.vector.scalar_tensor_tensor(
                        out_sb[:, osl], po0, recip_all[0][:, t:t + 1],
                        bo_full[:, osl], MULT, ADD,
                    )
                    po1 = ps.tile([128, 512], F32, tag="cont", bufs=3,
                                  name="po1")
                    nc.tensor.matmul(po1, aT[1][:, tsl], wo_h[1][:, osl],
                                     start=True, stop=True)
                    nc.vector.scalar_tensor_tensor(
                        out_sb[:, osl], po1, recip_all[1][:, t:t + 1],
                        out_sb[:, osl], MULT, ADD,
                    )
                nc.sync.dma_start(out=rsin[tsl, :], in_=out_sb)

            # ============ stage 5: ReduceScatter -> output slice ============
            nc.gpsimd.collective_compute(
                "ReduceScatter", ADD,
                replica_groups=GROUP,
                ins=[rsin.opt()], outs=[rsout.opt()],
            )
            nc.sync.dma_start(out=outp[:, :], in_=rsout[:])

            exit_stack_work.__exit__(None, None, None)

    _CACHED_NC = nc
    return nc


# ---------------------------------------------------------------------------
# Host wrapper
# ---------------------------------------------------------------------------
def _prep_inputs(q, k, v, pos_enc, Wq, bq, Wk, bk, Wv, bv, Wo, bo,
                 r_w_bias, r_r_bias, r_kernel):
    q2d = np.asarray(q, np.float32).reshape(L, D)
    k2d = np.asarray(k, np.float32).reshape(L, D)
    v2d = np.asarray(v, np.float32).reshape(L, D)
    p2d = np.asarray(pos_enc, np.float32)
    rwb = np.asarray(r_w_bias, np.float32).reshape(H, DH)
    rrb = np.asarray(r_r_bias, np.float32).reshape(H, DH)
    Wq = np.asarray(Wq, np.float32)
    Wk = np.asarray(Wk, np.float32)
    Wv = np.asarray(Wv, np.float32)
    Wo = np.asarray(Wo, np.float32)
    rkn = np.asarray(r_kernel, np.float32)
    bq = np.asarray(bq, np.float32)
    bk = np.asarray(bk, np.float32)
    bv = np.asarray(bv, np.float32)
    bo = np.asarray(bo, np.float32)

    in_maps = []
    for c in range(NCORES):
        sl = slice(DLOC * c, DLOC * (c + 1))
        hsl = slice(HPC * c, HPC * (c + 1))
        blob = np.zeros(TOT_E, NPBF16)
        act = blob[:ACT_E].reshape(128, ACT_W)
        act[:, 0:L] = q2d[:, sl].T
        act[:, L:2 * L] = k2d[:, sl].T
        act[:, 2 * L:3 * L] = v2d[:, sl].T
        act[:, 3 * L:3 * L + P_POS] = p2d[:, sl].T
        blob[WQOFF:WQOFF + W_E] = Wq[sl].T.astype(NPBF16).ravel()
        blob[WKOFF:WKOFF + W_E] = Wk[sl].T.astype(NPBF16).ravel()
        blob[WVOFF:WVOFF + W_E] = Wv[sl].T.astype(NPBF16).ravel()
        rk_c = rkn[hsl]  # [2, D, DH]
        blob[RKOFF:RKOFF + W_E] = np.concatenate(
            [rk_c[0], rk_c[1]], axis=1).astype(NPBF16).ravel()
        blob[WOOFF:WOOFF + W_E] = Wo[:, sl].T.astype(NPBF16).ravel()
        bq_c = bq[sl]
        blob[BQ1OFF:BQ1OFF + DLOC] = (
            SCALE * (bq_c + rwb[hsl].reshape(DLOC))).astype(NPBF16)
        blob[BQ2OFF:BQ2OFF + DLOC] = (
            SCALE * (bq_c + rrb[hsl].reshape(DLOC))).astype(NPBF16)
        blob[BKBOFF:BKBOFF + DLOC] = bk[sl].astype(NPBF16)
        blob[BVBOFF:BVBOFF + DLOC] = bv[sl].astype(NPBF16)
        if c == 0:
            blob[BOOFF:BOOFF + D] = bo.astype(NPBF16)
        in_maps.append({"blob": blob})
    return in_maps


# ---------------------------------------------------------------------------
# Cached sharded PJRT executable (built once per process)
# ---------------------------------------------------------------------------
_CACHED_FN = None
_CACHED_ZEROS = None


def _get_zeros():
    """Device-resident zero 'output' operands, reusable (not donated)."""
    global _CACHED_ZEROS
    if _CACHED_ZEROS is None:
        import jax
        _, _, _, zero_shapes, sharding = _get_fn()
        _CACHED_ZEROS = [
            jax.device_put(np.zeros((NCORES * s[0], *s[1:]), dt), sharding)
            for (s, dt) in zero_shapes
        ]
    return _CACHED_ZEROS


def _get_fn():
    global _CACHED_FN
    if _CACHED_FN is not None:
        return _CACHED_FN
    import jax
    from jax.sharding import Mesh, PartitionSpec
    from jax.experimental.shard_map import shard_map
    from concourse import bass2jax

    nc = build_program()
    bass2jax.install_neuronx_cc_hook()
    partition_name = (
        nc.partition_id_tensor.name if nc.partition_id_tensor else None
    )
    in_names, out_names, out_avals, zero_shapes = [], [], [], []
    for alloc in nc.m.functions[0].allocations:
        if not isinstance(alloc, mybir.MemoryLocationSet):
            continue
        name = alloc.memorylocations[0].name
        if alloc.kind == "ExternalInput":
            if name != partition_name:
                in_names.append(name)
        elif alloc.kind == "ExternalOutput":
            shape = tuple(alloc.tensor_shape)
            dtype = mybir.dt.np(alloc.dtype)
            out_names.append(name)
            out_avals.append(jax.core.ShapedArray(shape, dtype))
            zero_shapes.append((shape, dtype))
    n_params = len(in_names)
    n_outs = len(out_avals)
    all_in_names = list(in_names) + list(out_names)
    if partition_name is not None:
        all_in_names.append(partition_name)

    def _body(*args):
        operands = list(args)
        if partition_name is not None:
            operands.append(bass2jax.partition_id_tensor())
        outs = bass2jax._bass_exec_p.bind(
            *operands,
            out_avals=tuple(out_avals),
            in_names=tuple(all_in_names),
            out_names=tuple(out_names),
            lowering_input_output_aliases=(),
            sim_require_finite=True,
            sim_require_nnan=True,
            nc=nc,
        )
        return tuple(outs)

    devices = jax.devices()[:NCORES]
    mesh = Mesh(np.asarray(devices), ("core",))
    in_specs = (PartitionSpec("core"),) * (n_params + n_outs)
    out_specs = (PartitionSpec("core"),) * n_outs
    # No donation: outp is fully written by the program, so the zero
    # "output" operands are only read by the NEFF and can be reused
    # across calls. Skipping donation also measures ~1-2 ms faster.
    fn = jax.jit(
        shard_map(_body, mesh=mesh, in_specs=in_specs,
                  out_specs=out_specs, check_rep=False),
        keep_unused=True,
    )
    # pre-sharding args with this avoids an in-call reshard (~2-5 ms)
    from jax.sharding import NamedSharding
    sharding = NamedSharding(mesh, PartitionSpec("core"))
    _CACHED_FN = (fn, in_names, out_names, zero_shapes, sharding)
    return _CACHED_FN


def kernel(**inputs):
    global _CACHED_FN, _CACHED_ZEROS
    in_maps = _prep_inputs(**inputs)
    last_exc = None
    for attempt in range(2):
        try:
            import jax
            fn, in_names, out_names, zero_shapes, sharding = _get_fn()
            concat_in = [
                jax.device_put(
                    np.concatenate([m[name] for m in in_maps], axis=0),
                    sharding)
                for name in in_names
            ]
            outs = fn(*concat_in, *_get_zeros())
            out = np.asarray(outs[out_names.index("outp")])  # [L, D] f32
            return out.reshape(1, L, D).astype(np.float32)
        except Exception as e:  # transient axon/mesh hiccup: rebuild once
            last_exc = e
            _CACHED_FN = None
            _CACHED_ZEROS = None
    raise last_exc
